# revision 12
# baseline (speedup 1.0000x reference)
"""Trainium2 Bass kernel for GCE-TAGNN session recommendation model.

Strategy:
  - Vocab axis (10000 items, padded to 10240 = 8*1280) sharded across 8 cores
    for the global sparse aggregation and the target-attention score/softmax.
  - Session path data-parallel: 8 sessions per core; final/last/s_global
    all-gathered so every core has the full batch for target attention.
  - Target attention reformulated: with d = cand @ w3_W  ([N,384]),
      scores[b,n] = (sum_l E[b,l,n]*g[b,l,n]) / (sum_l E[b,l,n])
                    + last[b]·d[n,128:256] + s_global[b]·d[n,256:384]
    where ts[b,l,n] = final[b,l]·(w_target_W @ cand[n]), E = exp(ts) (no max
    subtraction needed: |ts| is small), g[b,l,n] = final[b,l]·d[n,:128].
    Padded (b,l) columns of final are zeroed, so E=1 and g=0 there; the
    softmax denominator is corrected by subtracting (L - len[b]).
"""

import sys

sys.path.insert(0, "/opt/trn_rl_repo")

import math

import ml_dtypes
import numpy as np

import concourse.bass as bass
import concourse.mybir as mybir
import concourse.tile as tile
from concourse import bacc
from concourse.bass import IndirectOffsetOnAxis
from concourse.bass_utils import run_bass_kernel_spmd

F32 = mybir.dt.float32
F32R = mybir.dt.float32r
BF16 = mybir.dt.bfloat16
I32 = mybir.dt.int32
AX = mybir.AxisListType
ALU = mybir.AluOpType
ACT = mybir.ActivationFunctionType

NC = 8          # cores
B = 64          # batch
L = 50          # session length
H = 128         # hidden
NH = 8          # heads
NIT = 10000     # item vocab
NPAD = NC * 1280  # padded vocab for candidate sharding
NS = 1280       # candidate shard per core
NCHUNK = NS // 128  # 10 n-chunks of 128 per core
BLOC = B // NC  # sessions per core
RL = BLOC * L   # 400 rows per core
WIN = 256       # agg row window
NWIN = 1280 // WIN  # 5 windows per core
BG = 8          # b-groups in phase D (each BLOC sessions = 400 cols)

USE_F32R = True


def _f32r(ap):
    return ap


_NC_CACHE = {}


def build_nc(T):
    """Build the single-NEFF SPMD program. T = edge tiles per window."""
    nc = bacc.Bacc(None, target_bir_lowering=False)

    def inp(name, shape, dtype=F32):
        return nc.dram_tensor(name, shape, dtype, kind="ExternalInput")

    # ---- replicated weights/constants ----
    embf = inp("embf", [NIT, H])
    posemb = inp("posemb", [200, H])
    idn = inp("idn", [H, H])
    iotab = inp("iotab", [H, WIN], BF16)
    blockdiag = inp("blockdiag", [H, NH])
    w_lin_inT = inp("w_lin_inT", [H, H])
    w_lin_outT = inp("w_lin_outT", [H, H])
    b_lin_in = inp("b_lin_in", [H, 1])
    b_lin_out = inp("b_lin_out", [H, 1])
    w_ihT = inp("w_ihT", [2 * H, 3 * H])
    w_hhT = inp("w_hhT", [H, 3 * H])
    b_ih = inp("b_ih", [3 * H, 1])
    b_hh = inp("b_hh", [3 * H, 1])
    in_projT = inp("in_projT", [H, 3 * H])
    in_projb = inp("in_projb", [3 * H, 1])
    out_projT = inp("out_projT", [H, H])
    out_projb = inp("out_projb", [H, 1])
    gWT = inp("gWT", [H, H])
    gb = inp("gb", [H, 1])
    w3 = inp("w3", [H, 3 * H])
    wtT = inp("wtT", [H, H])
    npadr = inp("npadr", [H, B])
    # ---- per-core ----
    adjT = inp("adjT", [BLOC, L, L])
    itemsx = inp("itemsx", [512, 1], I32)
    revx = inp("revx", [512, 1], I32)
    attmaskr = inp("attmaskr", [NH, RL])
    colmaskr = inp("colmaskr", [H, RL])
    lastselr = inp("lastselr", [H, RL])
    candT = inp("candT", [H, NS])
    eemb = inp("eemb", [H, NWIN * T, H], BF16)
    erowrel = inp("erowrel", [H, NWIN * T])
    ew = inp("ew", [H, NWIN * T])

    scores_out = nc.dram_tensor("scoresT", [NCHUNK, H, B], F32, kind="ExternalOutput")

    with tile.TileContext(nc) as tc:
        with (
            tc.tile_pool(name="cst", bufs=1) as cst,
            tc.tile_pool(name="wk", bufs=3) as wk,
            tc.tile_pool(name="pp", bufs=8, space="PSUM") as pp,
            tc.tile_pool(name="dr", bufs=1, space="DRAM") as dr,
        ):
            def psum(shape, tag="ps"):
                nbuf = {"ps": 3, "ts": 2, "gg": 2}[tag]
                return pp.tile(shape, F32, tag=tag, name=tag, bufs=nbuf)

            # ---------- load constants into SBUF ----------
            def load(name, src, shape=None, dtype=F32):
                t = cst.tile(shape if shape is not None else src.shape, dtype, name=name)
                nc.sync.dma_start(t[:], src[:])
                return t

            idn_sb = load("idn_sb", idn)
            idnb_sb = cst.tile([H, H], BF16, name="idnb_sb")
            nc.vector.tensor_copy(idnb_sb[:], idn_sb[:])
            iota_sb = load("iota_sb", iotab, dtype=BF16)
            bd_sb = load("bd_sb", blockdiag)
            linT_sb = load("linT_sb", w_lin_inT)
            loutT_sb = load("loutT_sb", w_lin_outT)
            blin_sb = load("blin_sb", b_lin_in)
            blout_sb = load("blout_sb", b_lin_out)
            wih_sb = cst.tile([H, 2, 3 * H], F32, name="wih_sb")
            nc.sync.dma_start(wih_sb[:], w_ihT.rearrange("(a p) j -> p a j", p=H))
            whh_sb = load("whh_sb", w_hhT)
            bih_sb = load("bih_sb", b_ih, shape=[H, 3])   # [384,1] -> [128,3]
            bhh_sb = load("bhh_sb", b_hh, shape=[H, 3])
            # reinterpret [384,1] dram as [128,3]: partition p, col g -> b[g*128+p]
            nc.sync.dma_start(bih_sb[:], b_ih.rearrange("(g p) o -> p (g o)", p=H))
            nc.sync.dma_start(bhh_sb[:], b_hh.rearrange("(g p) o -> p (g o)", p=H))
            prjT_sb = load("prjT_sb", in_projT)
            prjb_sb = cst.tile([H, 3], F32, name="prjb_sb")
            nc.sync.dma_start(prjb_sb[:], in_projb.rearrange("(g p) o -> p (g o)", p=H))
            oprjT_sb = load("oprjT_sb", out_projT)
            oprjb_sb = load("oprjb_sb", out_projb)
            gWT_f = load("gWT_f", gWT)
            gWT_sb = cst.tile([H, H], F32R, name="gWT_sb")
            nc.vector.tensor_copy(gWT_sb[:], gWT_f[:])
            gb_sb = load("gb_sb", gb)
            w3_f = load("w3_f", w3)
            w3_sb = cst.tile([H, 3 * H], F32R, name="w3_sb")
            nc.vector.tensor_copy(w3_sb[:], w3_f[:])
            wtT_f = load("wtT_f", wtT)
            wtT_sb = cst.tile([H, H], F32R, name="wtT_sb")
            nc.vector.tensor_copy(wtT_sb[:], wtT_f[:])
            npad_sb = load("npad_sb", npadr)
            am_sb = load("am_sb", attmaskr)
            cm_sb = load("cm_sb", colmaskr)
            ls_sb = load("ls_sb", lastselr)
            candT_f = load("candT_f", candT)
            candT_sb = cst.tile([H, NS], F32R, name="candT_sb")
            nc.vector.tensor_copy(candT_sb[:], candT_f[:])
            erow_sb = load("erow_sb", erowrel)
            ew_sb = load("ew_sb", ew)
            items_sb = cst.tile([H, 4], I32, name="items_sb")
            nc.sync.dma_start(items_sb[:], itemsx.rearrange("(t p) o -> p (t o)", p=H))
            rev_sb = cst.tile([H, 4], I32, name="rev_sb")
            nc.sync.dma_start(rev_sb[:], revx.rearrange("(t p) o -> p (t o)", p=H))

            # DRAM bounce buffers for collectives
            hg_shard = dr.tile([NS, H], BF16, name="hg_shard")
            hg_full = dr.tile([NC * NS, H], BF16, addr_space="Shared", name="hg_full")
            f_shard = dr.tile([H, RL], F32, name="f_shard")
            f_full = dr.tile([NC * H, RL], F32, addr_space="Shared", name="f_full")
            ls_shard = dr.tile([H, 2 * NH], F32, name="ls_shard")
            ls_full = dr.tile([NC * H, 2 * NH], F32, addr_space="Shared", name="ls_full")

            # =======================================================
            # Phase C: candidate transforms (independent of all else)
            # =======================================================
            cT = [cst.tile([H, NS], F32R, name=f"c{j}T") for j in range(3)]
            trT = cst.tile([H, NS], F32R, name="trT")
            nblk = [(0, 512), (512, 512), (1024, 256)]
            for j in range(3):
                for off, w in nblk:
                    ps = psum([H, w])
                    nc.tensor.matmul(
                        ps[:], _f32r(w3_sb[:, j * H:(j + 1) * H]),
                        _f32r(candT_sb[:, off:off + w]))
                    nc.vector.tensor_copy(cT[j][:, off:off + w], ps[:])
            for off, w in nblk:
                ps = psum([H, w])
                nc.tensor.matmul(ps[:], _f32r(wtT_sb[:]), _f32r(candT_sb[:, off:off + w]))
                nc.vector.tensor_copy(trT[:, off:off + w], ps[:])

            # =======================================================
            # Phase A: global GNN aggregation (vocab shard, 5 windows)
            # =======================================================
            aggT = cst.tile([H, NS], F32R, name="aggT")
            for w in range(NWIN):
                mt = wk.tile([H, T, H], BF16, tag="mt", bufs=2)
                nc.sync.dma_start(mt[:], eemb[:, w * T:(w + 1) * T, :])
                agg_ps = psum([H, WIN])
                for t in range(T):
                    j = w * T + t
                    sw = wk.tile([H, WIN], BF16, tag="sw")
                    nc.gpsimd.tensor_scalar(
                        out=sw[:], in0=iota_sb[:], scalar1=erow_sb[:, j:j + 1],
                        scalar2=ew_sb[:, j:j + 1], op0=ALU.is_equal, op1=ALU.mult)
                    nc.tensor.matmul(agg_ps[:], mt[:, t, :], sw[:],
                                     start=(t == 0), stop=(t == T - 1))
                nc.vector.tensor_copy(aggT[:, w * WIN:(w + 1) * WIN], agg_ps[:])
            # hgT = relu(gW @ agg + gb), stored bf16 for a cheaper all-gather
            hgT = cst.tile([H, NS], BF16, name="hgT")
            for off, w in nblk:
                ps = psum([H, w])
                nc.tensor.matmul(ps[:], _f32r(gWT_sb[:]), _f32r(aggT[:, off:off + w]))
                nc.scalar.activation(hgT[:, off:off + w], ps[:], ACT.Relu, bias=gb_sb[:, :1])
            # transpose to row-major [1280, 128] and store for all-gather
            hg_rm = cst.tile([H, NCHUNK, H], BF16, name="hg_rm")
            for k in range(NCHUNK):
                ps_b = pp.tile([H, H], BF16, tag="ps", name="ps_b", bufs=3)
                nc.tensor.transpose(ps_b[:], hgT[:, k * H:(k + 1) * H], idnb_sb[:])
                nc.vector.tensor_copy(hg_rm[:, k, :], ps_b[:])
            nc.sync.dma_start(hg_shard.rearrange("(k p) h -> p k h", p=H), hg_rm[:])
            nc.gpsimd.collective_compute(
                "AllGather", ALU.bypass, replica_groups=[list(range(NC))],
                ins=[hg_shard[:].opt()], outs=[hg_full[:].opt()])

            # =======================================================
            # Phase B: session path (8 local sessions)
            # =======================================================
            def gather_T(dst, table, idx_sb, tag, dtype=F32):
                """gather rows table[idx] -> transpose -> dst [128, 512]."""
                for t in range(4):
                    g = wk.tile([H, H], dtype, tag=tag)
                    nc.gpsimd.indirect_dma_start(
                        out=g[:], out_offset=None, in_=table[:, :],
                        in_offset=IndirectOffsetOnAxis(ap=idx_sb[:, t:t + 1], axis=0))
                    if dtype == BF16:
                        ps_g2 = pp.tile([H, H], BF16, tag="ps", name="ps_g2", bufs=3)
                        nc.tensor.transpose(ps_g2[:], g[:], idnb_sb[:])
                        nc.vector.tensor_copy(dst[:, t * H:(t + 1) * H], ps_g2[:])
                    else:
                        ps = psum([H, H])
                        nc.tensor.transpose(ps[:], g[:], idn_sb[:])
                        nc.vector.tensor_copy(dst[:, t * H:(t + 1) * H], ps[:])

            h0T = cst.tile([H, 512], F32, name="h0T")
            gather_T(h0T, embf, items_sb, "gh0")

            # Y = lin(h);  inp = adj @ Y   (per session)
            yinT = cst.tile([H, RL], F32, name="yinT")
            youtT = cst.tile([H, RL], F32, name="youtT")
            ps = psum([H, RL])
            nc.tensor.matmul(ps[:], _f32r(linT_sb[:]), _f32r(h0T[:, :RL]))
            nc.scalar.activation(yinT[:], ps[:], ACT.Identity, bias=blin_sb[:, :1])
            ps = psum([H, RL])
            nc.tensor.matmul(ps[:], _f32r(loutT_sb[:]), _f32r(h0T[:, :RL]))
            nc.scalar.activation(youtT[:], ps[:], ACT.Identity, bias=blout_sb[:, :1])

            iinT = cst.tile([H, RL], F32, name="iinT")
            ioutT = cst.tile([H, RL], F32, name="ioutT")
            for b in range(BLOC):
                at = wk.tile([L, L], F32, tag="at")
                nc.sync.dma_start(at[:], adjT[b])
                for yT, dst in ((yinT, iinT), (youtT, ioutT)):
                    ps_t = psum([L, H])
                    nc.tensor.transpose(ps_t[:], yT[:, b * L:(b + 1) * L], idn_sb[:])
                    yb = wk.tile([L, H], F32, tag="yb")
                    nc.vector.tensor_copy(yb[:], ps_t[:])
                    ps_i = psum([H, L], tag="ps")
                    nc.tensor.matmul(ps_i[:], yb[:], at[:])
                    nc.vector.tensor_copy(dst[:, b * L:(b + 1) * L], ps_i[:])

            # GRU cell (feature-major)
            combR = cst.tile([H, 2], F32, name="combR")
            nc.vector.tensor_add(combR[:, 0:1], bih_sb[:, 0:1], bhh_sb[:, 0:1])
            nc.vector.tensor_add(combR[:, 1:2], bih_sb[:, 1:2], bhh_sb[:, 1:2])
            gates = []
            for g in range(2):  # r, z
                ps_g = psum([H, RL])
                nc.tensor.matmul(ps_g[:], _f32r(wih_sb[:, 0, g * H:(g + 1) * H]),
                                 _f32r(iinT[:]), start=True, stop=False)
                nc.tensor.matmul(ps_g[:], _f32r(wih_sb[:, 1, g * H:(g + 1) * H]),
                                 _f32r(ioutT[:]), start=False, stop=False)
                nc.tensor.matmul(ps_g[:], _f32r(whh_sb[:, g * H:(g + 1) * H]),
                                 _f32r(h0T[:, :RL]), start=False, stop=True)
                gt = cst.tile([H, RL], F32, name=f"gate{g}")
                nc.scalar.activation(gt[:], ps_g[:], ACT.Sigmoid, bias=combR[:, g:g + 1])
                gates.append(gt)
            rT, zT = gates
            ps_in = psum([H, RL])
            nc.tensor.matmul(ps_in[:], _f32r(wih_sb[:, 0, 2 * H:3 * H]), _f32r(iinT[:]),
                             start=True, stop=False)
            nc.tensor.matmul(ps_in[:], _f32r(wih_sb[:, 1, 2 * H:3 * H]), _f32r(ioutT[:]),
                             start=False, stop=True)
            ps_hn = psum([H, RL])
            nc.tensor.matmul(ps_hn[:], _f32r(whh_sb[:, 2 * H:3 * H]), _f32r(h0T[:, :RL]))
            rhn = cst.tile([H, RL], F32, name="rhn")
            nc.vector.scalar_tensor_tensor(
                out=rhn[:], in0=ps_hn[:], scalar=bhh_sb[:, 2:3], in1=rT[:],
                op0=ALU.add, op1=ALU.mult)
            tmp_n = cst.tile([H, RL], F32, name="tmp_n")
            nc.vector.tensor_add(tmp_n[:], ps_in[:], rhn[:])
            nT = cst.tile([H, RL], F32, name="nT")
            nc.scalar.activation(nT[:], tmp_n[:], ACT.Tanh, bias=bih_sb[:, 2:3])
            diff = cst.tile([H, RL], F32, name="diff")
            nc.vector.tensor_sub(diff[:], h0T[:, :RL], nT[:])
            zd = cst.tile([H, RL], F32, name="zd")
            nc.vector.tensor_mul(zd[:], zT[:], diff[:])
            h1T = cst.tile([H, RL], F32, name="h1T")
            nc.vector.tensor_add(h1T[:], nT[:], zd[:])

            # rich = hg[items] + h1; final = (rich + pos_emb[rev]) * colmask
            sgT = cst.tile([H, 512], BF16, name="sgT")
            gather_T(sgT, hg_full, items_sb, "gsg", dtype=BF16)
            poT = cst.tile([H, 512], F32, name="poT")
            gather_T(poT, posemb, rev_sb, "gpo")
            richT = cst.tile([H, RL], F32, name="richT")
            nc.vector.tensor_add(richT[:], h1T[:], sgT[:, :RL])
            finT = cst.tile([H, RL], F32, name="finT")
            nc.vector.tensor_add(finT[:], richT[:], poT[:, :RL])
            nc.vector.tensor_mul(finT[:], finT[:], cm_sb[:])

            # last[b] = final[b, len_b - 1]  (one-hot selection + reduce)
            lsel = cst.tile([H, RL], F32, name="lsel")
            nc.vector.tensor_mul(lsel[:], finT[:], ls_sb[:])
            lastT = cst.tile([H, NH], F32, name="lastT")
            nc.vector.reduce_sum(lastT[:], lsel[:].rearrange("p (b l) -> p b l", b=BLOC),
                                 axis=AX.X)

            # ---- multi-head attention (q = last, kv = final) ----
            qT = cst.tile([H, NH], F32, name="qT")
            ps_q = psum([H, NH])
            nc.tensor.matmul(ps_q[:], _f32r(prjT_sb[:, 0:H]), _f32r(lastT[:]))
            nc.scalar.activation(qT[:], ps_q[:], ACT.Identity, bias=prjb_sb[:, 0:1])
            kT = cst.tile([H, RL], F32, name="kT")
            ps_k = psum([H, RL])
            nc.tensor.matmul(ps_k[:], _f32r(prjT_sb[:, H:2 * H]), _f32r(finT[:]))
            nc.scalar.activation(kT[:], ps_k[:], ACT.Identity, bias=prjb_sb[:, 1:2])
            vT = cst.tile([H, RL], F32, name="vT")
            ps_v = psum([H, RL])
            nc.tensor.matmul(ps_v[:], _f32r(prjT_sb[:, 2 * H:3 * H]), _f32r(finT[:]))
            nc.scalar.activation(vT[:], ps_v[:], ACT.Identity, bias=prjb_sb[:, 2:3])

            ctxT = cst.tile([H, NH], F32, name="ctxT")
            for b in range(BLOC):
                qb = wk.tile([H, NH], F32, tag="qb")
                nc.vector.tensor_mul(qb[:], qT[:, b:b + 1].to_broadcast([H, NH]), bd_sb[:])
                ps_a = psum([NH, L], tag="ps")
                nc.tensor.matmul(ps_a[:], qb[:], kT[:, b * L:(b + 1) * L])
                attm = wk.tile([NH, L], F32, tag="attm")
                nc.vector.tensor_add(attm[:], ps_a[:], am_sb[:, b * L:(b + 1) * L])
                negmax = wk.tile([NH, 1], F32, tag="negmax")
                nc.vector.tensor_reduce(negmax[:], attm[:], axis=AX.X, op=ALU.max,
                                        negate=True)
                attE = wk.tile([NH, L], F32, tag="attE")
                den_a = wk.tile([NH, 1], F32, tag="den_a")
                nc.scalar.activation(attE[:], attm[:], ACT.Exp, bias=negmax[:, :1],
                                     accum_out=den_a[:, :1])
                rec_a = wk.tile([NH, 1], F32, tag="rec_a")
                nc.vector.reciprocal(rec_a[:], den_a[:])
                attw = wk.tile([NH, L], F32, tag="attw")
                nc.vector.tensor_scalar_mul(attw[:], attE[:], rec_a[:, :1])
                ps_wt = psum([L, NH])
                nc.tensor.transpose(ps_wt[:], attw[:], idn_sb[:NH, :NH])
                awT = wk.tile([L, NH], F32, tag="awT")
                nc.vector.tensor_copy(awT[:], ps_wt[:])
                ps_vt = psum([L, H])
                nc.tensor.transpose(ps_vt[:], vT[:, b * L:(b + 1) * L], idn_sb[:])
                vb = wk.tile([L, H], F32, tag="vb")
                nc.vector.tensor_copy(vb[:], ps_vt[:])
                ps_o = psum([H, NH], tag="ps")
                nc.tensor.matmul(ps_o[:], vb[:], awT[:])
                o2 = wk.tile([H, NH], F32, tag="o2")
                nc.vector.tensor_mul(o2[:], ps_o[:], bd_sb[:])
                nc.vector.reduce_sum(ctxT[:, b:b + 1], o2[:], axis=AX.X)

            sgloT = cst.tile([H, NH], F32, name="sgloT")
            ps_sg = psum([H, NH])
            nc.tensor.matmul(ps_sg[:], _f32r(oprjT_sb[:]), _f32r(ctxT[:]))
            nc.scalar.activation(sgloT[:], ps_sg[:], ACT.Identity, bias=oprjb_sb[:, :1])

            # ---- all-gather final / (last, s_global) ----
            nc.sync.dma_start(f_shard[:], finT[:])
            nc.gpsimd.collective_compute(
                "AllGather", ALU.bypass, replica_groups=[list(range(NC))],
                ins=[f_shard[:].opt()], outs=[f_full[:].opt()])
            lspair = cst.tile([H, 2 * NH], F32, name="lspair")
            nc.vector.tensor_copy(lspair[:, 0:NH], lastT[:])
            nc.vector.tensor_copy(lspair[:, NH:2 * NH], sgloT[:])
            nc.sync.dma_start(ls_shard[:], lspair[:])
            nc.gpsimd.collective_compute(
                "AllGather", ALU.bypass, replica_groups=[list(range(NC))],
                ins=[ls_shard[:].opt()], outs=[ls_full[:].opt()])

            fullT_f = cst.tile([H, B * L], F32, name="fullT_f")
            nc.sync.dma_start(fullT_f[:].rearrange("p (c r) -> p c r", c=NC),
                              f_full.rearrange("(c p) r -> p c r", p=H))
            fullT = cst.tile([H, B * L], F32R, name="fullT")
            nc.vector.tensor_copy(fullT[:], fullT_f[:])
            lastF_f = cst.tile([H, B], F32, name="lastF_f")
            sglF_f = cst.tile([H, B], F32, name="sglF_f")
            lsv = ls_full.rearrange("(c p) x -> p c x", p=H)
            nc.sync.dma_start(lastF_f[:].rearrange("p (c x) -> p c x", c=NC),
                              lsv[:, :, 0:NH])
            nc.sync.dma_start(sglF_f[:].rearrange("p (c x) -> p c x", c=NC),
                              lsv[:, :, NH:2 * NH])
            lastF = cst.tile([H, B], F32R, name="lastF")
            nc.vector.tensor_copy(lastF[:], lastF_f[:])
            sglF = cst.tile([H, B], F32R, name="sglF")
            nc.vector.tensor_copy(sglF[:], sglF_f[:])

            # =======================================================
            # Phase D: target attention over the candidate shard
            # =======================================================
            GW = RL  # 400 columns per b-group
            for ch in range(NCHUNK):
                num = wk.tile([H, B], F32, tag="num")
                den = wk.tile([H, B], F32, tag="den")
                eT = wk.tile([H, B * L], F32, tag="eT", bufs=2)
                pT = wk.tile([H, B * L], F32, tag="pT", bufs=2)
                for bg in range(BG):
                    rhs = fullT[:, bg * GW:(bg + 1) * GW]
                    ps_ts = psum([H, GW], tag="ts")
                    nc.tensor.matmul(ps_ts[:], _f32r(trT[:, ch * H:(ch + 1) * H]),
                                     _f32r(rhs))
                    ps_g = psum([H, GW], tag="gg")
                    nc.tensor.matmul(ps_g[:], _f32r(cT[0][:, ch * H:(ch + 1) * H]),
                                     _f32r(rhs))
                    nc.scalar.activation(eT[:, bg * GW:(bg + 1) * GW], ps_ts[:], ACT.Exp)
                    nc.vector.tensor_mul(pT[:, bg * GW:(bg + 1) * GW],
                                         eT[:, bg * GW:(bg + 1) * GW], ps_g[:])
                nc.vector.reduce_sum(den[:], eT[:].rearrange("p (b l) -> p b l", b=B),
                                     axis=AX.X)
                nc.vector.reduce_sum(num[:], pT[:].rearrange("p (b l) -> p b l", b=B),
                                     axis=AX.X)
                denf = wk.tile([H, B], F32, tag="denf")
                nc.vector.tensor_sub(denf[:], den[:], npad_sb[:])
                rec = wk.tile([H, B], F32, tag="rec")
                nc.vector.reciprocal(rec[:], denf[:])
                t1 = wk.tile([H, B], F32, tag="t1")
                nc.vector.tensor_mul(t1[:], num[:], rec[:])
                ps_23 = psum([H, B])
                nc.tensor.matmul(ps_23[:], _f32r(cT[1][:, ch * H:(ch + 1) * H]),
                                 _f32r(lastF[:]), start=True, stop=False)
                nc.tensor.matmul(ps_23[:], _f32r(cT[2][:, ch * H:(ch + 1) * H]),
                                 _f32r(sglF[:]), start=False, stop=True)
                outT = wk.tile([H, B], F32, tag="outT")
                nc.vector.tensor_add(outT[:], t1[:], ps_23[:])
                nc.sync.dma_start(scores_out[ch], outT[:])

    nc.compile()
    return nc


# ==============================================================
# Host side: shard inputs, run, gather output
# ==============================================================

def _prep(inputs):
    """Build per-core input maps (numpy only: layout/sharding/index prep)."""
    emb = np.asarray(inputs["emb"], np.float32)
    items = np.asarray(inputs["session_items"], np.int32)
    lens = np.asarray(inputs["session_len"], np.int32)
    adj = np.asarray(inputs["session_adj"], np.float32)
    erow = np.asarray(inputs["global_edge_row"], np.int32)
    ecol_g = np.asarray(inputs["global_edge_col"], np.int32)
    ew_g = np.asarray(inputs["global_edge_weight"], np.float32)

    rep = {}
    rep["embf"] = emb
    embb = emb.astype(ml_dtypes.bfloat16)
    rep["posemb"] = np.asarray(inputs["pos_emb"], np.float32)
    rep["idn"] = np.eye(H, dtype=np.float32)
    rep["iotab"] = np.broadcast_to(
        np.arange(WIN, dtype=np.float32), (H, WIN)).astype(ml_dtypes.bfloat16).copy()
    rep["blockdiag"] = np.kron(np.eye(NH, dtype=np.float32),
                               np.ones((H // NH, 1), np.float32))
    rep["w_lin_inT"] = np.ascontiguousarray(np.asarray(inputs["lin_in_W"], np.float32).T)
    rep["w_lin_outT"] = np.ascontiguousarray(np.asarray(inputs["lin_out_W"], np.float32).T)
    rep["b_lin_in"] = np.asarray(inputs["lin_in_b"], np.float32).reshape(H, 1)
    rep["b_lin_out"] = np.asarray(inputs["lin_out_b"], np.float32).reshape(H, 1)
    rep["w_ihT"] = np.ascontiguousarray(np.asarray(inputs["w_ih"], np.float32).T)
    rep["w_hhT"] = np.ascontiguousarray(np.asarray(inputs["w_hh"], np.float32).T)
    rep["b_ih"] = np.asarray(inputs["b_ih"], np.float32).reshape(3 * H, 1)
    rep["b_hh"] = np.asarray(inputs["b_hh"], np.float32).reshape(3 * H, 1)
    ipw = np.asarray(inputs["in_proj_w"], np.float32).copy()
    ipb = np.asarray(inputs["in_proj_b"], np.float32).copy()
    scale = 1.0 / math.sqrt(H // NH)
    ipw[:H] *= scale
    ipb[:H] *= scale
    rep["in_projT"] = np.ascontiguousarray(ipw.T)
    rep["in_projb"] = ipb.reshape(3 * H, 1)
    rep["out_projT"] = np.ascontiguousarray(np.asarray(inputs["out_proj_w"], np.float32).T)
    rep["out_projb"] = np.asarray(inputs["out_proj_b"], np.float32).reshape(H, 1)
    rep["gWT"] = np.ascontiguousarray(np.asarray(inputs["gW"], np.float32).T)
    rep["gb"] = np.asarray(inputs["gb"], np.float32).reshape(H, 1)
    rep["w3"] = np.asarray(inputs["w3_W"], np.float32)
    rep["wtT"] = np.ascontiguousarray(np.asarray(inputs["w_target_W"], np.float32).T)
    rep["npadr"] = np.broadcast_to((L - lens).astype(np.float32), (H, B)).copy()

    # --- global edges: sort by row, shard by vocab range, window-pack ---
    order = np.argsort(erow, kind="stable")
    erow_s, ecol_s, ew_s = erow[order], ecol_g[order], ew_g[order]
    # window id = row // WIN  (NC*NWIN = 40 windows over padded vocab)
    nwin_tot = NC * NWIN
    win_id = erow_s // WIN
    counts = np.bincount(win_id, minlength=nwin_tot)
    T = max(1, int(math.ceil(counts.max() / H)))
    starts = np.zeros(nwin_tot + 1, np.int64)
    np.cumsum(counts, out=starts[1:])

    cand_full = np.zeros((NPAD, H), np.float32)
    cand_full[:NIT - 1] = emb[1:]

    per_core = []
    for c in range(NC):
        ec = np.zeros((NWIN * T * H,), np.int32)
        er = np.full((NWIN * T * H,), 300.0, np.float32)
        evw = np.zeros((NWIN * T * H,), np.float32)
        for w in range(NWIN):
            gw = c * NWIN + w
            s, e = starts[gw], starts[gw + 1]
            n = e - s
            ec[w * T * H: w * T * H + n] = ecol_s[s:e]
            er[w * T * H: w * T * H + n] = (erow_s[s:e] - gw * WIN).astype(np.float32)
            evw[w * T * H: w * T * H + n] = ew_s[s:e]
        # [NWIN*T*H] -> [H, NWIN*T]: tile j, partition p <- j*H + p
        ec2 = ec.reshape(NWIN * T, H).T
        er2 = er.reshape(NWIN * T, H).T
        ev2 = evw.reshape(NWIN * T, H).T

        bsl = slice(c * BLOC, (c + 1) * BLOC)
        it_loc = items[bsl]                      # [8, 50]
        len_loc = lens[bsl]
        pos_idx = np.arange(L)[None, :]
        rev = len_loc[:, None] - 1 - pos_idx
        rev = np.where(it_loc == 0, 0, rev).astype(np.int32)
        pad = (it_loc == 0)

        itemsx = np.zeros((512, 1), np.int32)
        itemsx[:RL, 0] = it_loc.reshape(-1)
        revx = np.zeros((512, 1), np.int32)
        revx[:RL, 0] = rev.reshape(-1)
        attmask = np.where(pad, -1e9, 0.0).astype(np.float32).reshape(1, RL)
        colmask = (~pad).astype(np.float32).reshape(1, RL)
        lastsel = np.zeros((BLOC, L), np.float32)
        lastsel[np.arange(BLOC), len_loc - 1] = 1.0

        m = dict(rep)
        m["adjT"] = np.ascontiguousarray(adj[bsl].transpose(0, 2, 1))
        m["itemsx"] = itemsx
        m["revx"] = revx
        m["attmaskr"] = np.broadcast_to(attmask, (NH, RL)).copy()
        m["colmaskr"] = np.broadcast_to(colmask, (H, RL)).copy()
        m["lastselr"] = np.broadcast_to(lastsel.reshape(1, RL), (H, RL)).copy()
        m["candT"] = np.ascontiguousarray(cand_full[c * NS:(c + 1) * NS].T)
        m["eemb"] = np.ascontiguousarray(embb[ec2])
        m["erowrel"] = np.ascontiguousarray(er2)
        m["ew"] = np.ascontiguousarray(ev2)
        per_core.append(m)
    return per_core, T


def kernel(_trace=False, **inputs):
    in_maps, T = _prep(inputs)
    if T not in _NC_CACHE:
        _NC_CACHE[T] = build_nc(T)
    nc = _NC_CACHE[T]
    res = run_bass_kernel_spmd(nc, in_maps, core_ids=list(range(NC)),
                               trace=_trace)
    scores = np.concatenate(
        [res.results[c]["scoresT"].transpose(2, 0, 1).reshape(B, NS)
         for c in range(NC)], axis=1)[:, :NIT - 1]
    if _trace:
        return scores, res
    return scores


# revision 13
# speedup vs baseline: 2.9499x; 2.9499x over previous
"""Trainium2 Bass kernel for GCE-TAGNN session recommendation model.

Strategy:
  - Vocab axis (10000 items, padded to 10240 = 8*1280) sharded across 8 cores
    for the global sparse aggregation and the target-attention score/softmax.
  - Session path data-parallel: 8 sessions per core; final/last/s_global
    all-gathered so every core has the full batch for target attention.
  - Target attention reformulated: with d = cand @ w3_W  ([N,384]),
      scores[b,n] = (sum_l E[b,l,n]*g[b,l,n]) / (sum_l E[b,l,n])
                    + last[b]·d[n,128:256] + s_global[b]·d[n,256:384]
    where ts[b,l,n] = final[b,l]·(w_target_W @ cand[n]), E = exp(ts) (no max
    subtraction needed: |ts| is small), g[b,l,n] = final[b,l]·d[n,:128].
    Padded (b,l) columns of final are zeroed, so E=1 and g=0 there; the
    softmax denominator is corrected by subtracting (L - len[b]).
"""

import sys

sys.path.insert(0, "/opt/trn_rl_repo")

import math

import ml_dtypes
import numpy as np

import concourse.bass as bass
import concourse.mybir as mybir
import concourse.tile as tile
from concourse import bacc
from concourse.bass import IndirectOffsetOnAxis
from concourse.bass_utils import run_bass_kernel_spmd

F32 = mybir.dt.float32
F32R = mybir.dt.float32r
BF16 = mybir.dt.bfloat16
I32 = mybir.dt.int32
AX = mybir.AxisListType
ALU = mybir.AluOpType
ACT = mybir.ActivationFunctionType

NC = 8          # cores
B = 64          # batch
L = 50          # session length
H = 128         # hidden
NH = 8          # heads
NIT = 10000     # item vocab
NPAD = NC * 1280  # padded vocab for candidate sharding
NS = 1280       # candidate shard per core
NCHUNK = NS // 128  # 10 n-chunks of 128 per core
BLOC = B // NC  # sessions per core
RL = BLOC * L   # 400 rows per core
WIN = 256       # agg row window
NWIN = 1280 // WIN  # 5 windows per core
BG = 8          # b-groups in phase D (each BLOC sessions = 400 cols)

USE_F32R = True


def _f32r(ap):
    return ap


_NC_CACHE = {}


def build_nc(T):
    """Build the single-NEFF SPMD program. T = edge tiles per window."""
    nc = bacc.Bacc(None, target_bir_lowering=False)

    def inp(name, shape, dtype=F32):
        return nc.dram_tensor(name, shape, dtype, kind="ExternalInput")

    # ---- replicated weights/constants ----
    embf = inp("embf", [NIT, H])
    posemb = inp("posemb", [200, H])
    idn = inp("idn", [H, H])
    iotab = inp("iotab", [H, WIN], BF16)
    blockdiag = inp("blockdiag", [H, NH])
    w_lin_inT = inp("w_lin_inT", [H, H])
    w_lin_outT = inp("w_lin_outT", [H, H])
    b_lin_in = inp("b_lin_in", [H, 1])
    b_lin_out = inp("b_lin_out", [H, 1])
    w_ihT = inp("w_ihT", [2 * H, 3 * H])
    w_hhT = inp("w_hhT", [H, 3 * H])
    b_ih = inp("b_ih", [3 * H, 1])
    b_hh = inp("b_hh", [3 * H, 1])
    in_projT = inp("in_projT", [H, 3 * H])
    in_projb = inp("in_projb", [3 * H, 1])
    out_projT = inp("out_projT", [H, H])
    out_projb = inp("out_projb", [H, 1])
    gWT = inp("gWT", [H, H])
    gb = inp("gb", [H, 1])
    w3 = inp("w3", [H, 3 * H])
    wtT = inp("wtT", [H, H])
    npadr = inp("npadr", [H, B])
    # ---- per-core ----
    adjT = inp("adjT", [BLOC, L, L])
    itemsx = inp("itemsx", [512, 1], I32)
    revx = inp("revx", [512, 1], I32)
    attmaskr = inp("attmaskr", [NH, RL])
    colmaskr = inp("colmaskr", [H, RL])
    lastselr = inp("lastselr", [H, RL])
    candT = inp("candT", [H, NS])
    eemb = inp("eemb", [H, NWIN * T, H], BF16)
    erowrel = inp("erowrel", [H, NWIN * T])
    ew = inp("ew", [H, NWIN * T])

    scores_out = nc.dram_tensor("scoresT", [NCHUNK, H, B], F32, kind="ExternalOutput")

    with tile.TileContext(nc) as tc:
        with (
            tc.tile_pool(name="cst", bufs=1) as cst,
            tc.tile_pool(name="wk", bufs=3) as wk,
            tc.tile_pool(name="pp", bufs=8, space="PSUM") as pp,
            tc.tile_pool(name="dr", bufs=1, space="DRAM") as dr,
        ):
            def psum(shape, tag="ps"):
                nbuf = {"ps": 3, "ts": 2, "gg": 2}[tag]
                return pp.tile(shape, F32, tag=tag, name=tag, bufs=nbuf)

            # ---------- load constants into SBUF ----------
            def load(name, src, shape=None, dtype=F32):
                t = cst.tile(shape if shape is not None else src.shape, dtype, name=name)
                nc.sync.dma_start(t[:], src[:])
                return t

            idn_sb = load("idn_sb", idn)
            idnb_sb = cst.tile([H, H], BF16, name="idnb_sb")
            nc.vector.tensor_copy(idnb_sb[:], idn_sb[:])
            iota_sb = load("iota_sb", iotab, dtype=BF16)
            bd_sb = load("bd_sb", blockdiag)
            linT_sb = load("linT_sb", w_lin_inT)
            loutT_sb = load("loutT_sb", w_lin_outT)
            blin_sb = load("blin_sb", b_lin_in)
            blout_sb = load("blout_sb", b_lin_out)
            wih_sb = cst.tile([H, 2, 3 * H], F32, name="wih_sb")
            nc.sync.dma_start(wih_sb[:], w_ihT.rearrange("(a p) j -> p a j", p=H))
            whh_sb = load("whh_sb", w_hhT)
            bih_sb = load("bih_sb", b_ih, shape=[H, 3])   # [384,1] -> [128,3]
            bhh_sb = load("bhh_sb", b_hh, shape=[H, 3])
            # reinterpret [384,1] dram as [128,3]: partition p, col g -> b[g*128+p]
            nc.sync.dma_start(bih_sb[:], b_ih.rearrange("(g p) o -> p (g o)", p=H))
            nc.sync.dma_start(bhh_sb[:], b_hh.rearrange("(g p) o -> p (g o)", p=H))
            prjT_sb = load("prjT_sb", in_projT)
            prjb_sb = cst.tile([H, 3], F32, name="prjb_sb")
            nc.sync.dma_start(prjb_sb[:], in_projb.rearrange("(g p) o -> p (g o)", p=H))
            oprjT_sb = load("oprjT_sb", out_projT)
            oprjb_sb = load("oprjb_sb", out_projb)
            gWT_f = load("gWT_f", gWT)
            gWT_sb = cst.tile([H, H], F32R, name="gWT_sb")
            nc.vector.tensor_copy(gWT_sb[:], gWT_f[:])
            gb_sb = load("gb_sb", gb)
            w3_f = load("w3_f", w3)
            w3_sb = cst.tile([H, 3 * H], F32R, name="w3_sb")
            nc.vector.tensor_copy(w3_sb[:], w3_f[:])
            wtT_f = load("wtT_f", wtT)
            wtT_sb = cst.tile([H, H], F32R, name="wtT_sb")
            nc.vector.tensor_copy(wtT_sb[:], wtT_f[:])
            npad_sb = load("npad_sb", npadr)
            am_sb = load("am_sb", attmaskr)
            cm_sb = load("cm_sb", colmaskr)
            ls_sb = load("ls_sb", lastselr)
            candT_f = load("candT_f", candT)
            candT_sb = cst.tile([H, NS], F32R, name="candT_sb")
            nc.vector.tensor_copy(candT_sb[:], candT_f[:])
            erow_sb = load("erow_sb", erowrel)
            ew_sb = load("ew_sb", ew)
            items_sb = cst.tile([H, 4], I32, name="items_sb")
            nc.sync.dma_start(items_sb[:], itemsx.rearrange("(t p) o -> p (t o)", p=H))
            rev_sb = cst.tile([H, 4], I32, name="rev_sb")
            nc.sync.dma_start(rev_sb[:], revx.rearrange("(t p) o -> p (t o)", p=H))

            # DRAM bounce buffers for collectives
            hg_shard = dr.tile([NS, H], BF16, name="hg_shard")
            hg_full = dr.tile([NC * NS, H], BF16, addr_space="Shared", name="hg_full")
            f_shard = dr.tile([H, RL], F32, name="f_shard")
            f_full = dr.tile([NC * H, RL], F32, addr_space="Shared", name="f_full")
            ls_shard = dr.tile([H, 2 * NH], F32, name="ls_shard")
            ls_full = dr.tile([NC * H, 2 * NH], F32, addr_space="Shared", name="ls_full")

            # =======================================================
            # Phase C: candidate transforms (independent of all else)
            # =======================================================
            cT = [cst.tile([H, NS], F32R, name=f"c{j}T") for j in range(3)]
            trT = cst.tile([H, NS], F32R, name="trT")
            nblk = [(0, 512), (512, 512), (1024, 256)]
            for j in range(3):
                for off, w in nblk:
                    ps = psum([H, w])
                    nc.tensor.matmul(
                        ps[:], _f32r(w3_sb[:, j * H:(j + 1) * H]),
                        _f32r(candT_sb[:, off:off + w]))
                    nc.vector.tensor_copy(cT[j][:, off:off + w], ps[:])
            for off, w in nblk:
                ps = psum([H, w])
                nc.tensor.matmul(ps[:], _f32r(wtT_sb[:]), _f32r(candT_sb[:, off:off + w]))
                nc.vector.tensor_copy(trT[:, off:off + w], ps[:])

            # =======================================================
            # Phase A: global GNN aggregation (vocab shard, 5 windows)
            # =======================================================
            aggT = cst.tile([H, NS], F32R, name="aggT")
            for w in range(NWIN):
                mt = wk.tile([H, T, H], BF16, tag="mt", bufs=2)
                nc.sync.dma_start(mt[:], eemb[:, w * T:(w + 1) * T, :])
                agg_ps = psum([H, WIN])
                for t in range(T):
                    j = w * T + t
                    sw = wk.tile([H, WIN], BF16, tag="sw")
                    nc.vector.tensor_scalar(
                        out=sw[:], in0=iota_sb[:], scalar1=erow_sb[:, j:j + 1],
                        scalar2=ew_sb[:, j:j + 1], op0=ALU.is_equal, op1=ALU.mult)
                    nc.tensor.matmul(agg_ps[:], mt[:, t, :], sw[:],
                                     start=(t == 0), stop=(t == T - 1))
                nc.vector.tensor_copy(aggT[:, w * WIN:(w + 1) * WIN], agg_ps[:])
            # hgT = relu(gW @ agg + gb), stored bf16 for a cheaper all-gather
            hgT = cst.tile([H, NS], BF16, name="hgT")
            for off, w in nblk:
                ps = psum([H, w])
                nc.tensor.matmul(ps[:], _f32r(gWT_sb[:]), _f32r(aggT[:, off:off + w]))
                nc.scalar.activation(hgT[:, off:off + w], ps[:], ACT.Relu, bias=gb_sb[:, :1])
            # transpose to row-major [1280, 128] and store for all-gather
            hg_rm = cst.tile([H, NCHUNK, H], BF16, name="hg_rm")
            for k in range(NCHUNK):
                ps_b = pp.tile([H, H], BF16, tag="ps", name="ps_b", bufs=3)
                nc.tensor.transpose(ps_b[:], hgT[:, k * H:(k + 1) * H], idnb_sb[:])
                nc.vector.tensor_copy(hg_rm[:, k, :], ps_b[:])
            nc.sync.dma_start(hg_shard.rearrange("(k p) h -> p k h", p=H), hg_rm[:])
            nc.gpsimd.collective_compute(
                "AllGather", ALU.bypass, replica_groups=[list(range(NC))],
                ins=[hg_shard[:].opt()], outs=[hg_full[:].opt()])

            # =======================================================
            # Phase B: session path (8 local sessions)
            # =======================================================
            def gather_T(dst, table, idx_sb, tag, dtype=F32):
                """gather rows table[idx] -> transpose -> dst [128, 512]."""
                for t in range(4):
                    g = wk.tile([H, H], dtype, tag=tag)
                    nc.gpsimd.indirect_dma_start(
                        out=g[:], out_offset=None, in_=table[:, :],
                        in_offset=IndirectOffsetOnAxis(ap=idx_sb[:, t:t + 1], axis=0))
                    if dtype == BF16:
                        ps_g2 = pp.tile([H, H], BF16, tag="ps", name="ps_g2", bufs=3)
                        nc.tensor.transpose(ps_g2[:], g[:], idnb_sb[:])
                        nc.vector.tensor_copy(dst[:, t * H:(t + 1) * H], ps_g2[:])
                    else:
                        ps = psum([H, H])
                        nc.tensor.transpose(ps[:], g[:], idn_sb[:])
                        nc.vector.tensor_copy(dst[:, t * H:(t + 1) * H], ps[:])

            h0T = cst.tile([H, 512], F32, name="h0T")
            gather_T(h0T, embf, items_sb, "gh0")

            # Y = lin(h);  inp = adj @ Y   (per session)
            yinT = cst.tile([H, RL], F32, name="yinT")
            youtT = cst.tile([H, RL], F32, name="youtT")
            ps = psum([H, RL])
            nc.tensor.matmul(ps[:], _f32r(linT_sb[:]), _f32r(h0T[:, :RL]))
            nc.scalar.activation(yinT[:], ps[:], ACT.Identity, bias=blin_sb[:, :1])
            ps = psum([H, RL])
            nc.tensor.matmul(ps[:], _f32r(loutT_sb[:]), _f32r(h0T[:, :RL]))
            nc.scalar.activation(youtT[:], ps[:], ACT.Identity, bias=blout_sb[:, :1])

            iinT = cst.tile([H, RL], F32, name="iinT")
            ioutT = cst.tile([H, RL], F32, name="ioutT")
            for b in range(BLOC):
                at = wk.tile([L, L], F32, tag="at")
                nc.sync.dma_start(at[:], adjT[b])
                for yT, dst in ((yinT, iinT), (youtT, ioutT)):
                    ps_t = psum([L, H])
                    nc.tensor.transpose(ps_t[:], yT[:, b * L:(b + 1) * L], idn_sb[:])
                    yb = wk.tile([L, H], F32, tag="yb")
                    nc.vector.tensor_copy(yb[:], ps_t[:])
                    ps_i = psum([H, L], tag="ps")
                    nc.tensor.matmul(ps_i[:], yb[:], at[:])
                    nc.vector.tensor_copy(dst[:, b * L:(b + 1) * L], ps_i[:])

            # GRU cell (feature-major)
            combR = cst.tile([H, 2], F32, name="combR")
            nc.vector.tensor_add(combR[:, 0:1], bih_sb[:, 0:1], bhh_sb[:, 0:1])
            nc.vector.tensor_add(combR[:, 1:2], bih_sb[:, 1:2], bhh_sb[:, 1:2])
            gates = []
            for g in range(2):  # r, z
                ps_g = psum([H, RL])
                nc.tensor.matmul(ps_g[:], _f32r(wih_sb[:, 0, g * H:(g + 1) * H]),
                                 _f32r(iinT[:]), start=True, stop=False)
                nc.tensor.matmul(ps_g[:], _f32r(wih_sb[:, 1, g * H:(g + 1) * H]),
                                 _f32r(ioutT[:]), start=False, stop=False)
                nc.tensor.matmul(ps_g[:], _f32r(whh_sb[:, g * H:(g + 1) * H]),
                                 _f32r(h0T[:, :RL]), start=False, stop=True)
                gt = cst.tile([H, RL], F32, name=f"gate{g}")
                nc.scalar.activation(gt[:], ps_g[:], ACT.Sigmoid, bias=combR[:, g:g + 1])
                gates.append(gt)
            rT, zT = gates
            ps_in = psum([H, RL])
            nc.tensor.matmul(ps_in[:], _f32r(wih_sb[:, 0, 2 * H:3 * H]), _f32r(iinT[:]),
                             start=True, stop=False)
            nc.tensor.matmul(ps_in[:], _f32r(wih_sb[:, 1, 2 * H:3 * H]), _f32r(ioutT[:]),
                             start=False, stop=True)
            ps_hn = psum([H, RL])
            nc.tensor.matmul(ps_hn[:], _f32r(whh_sb[:, 2 * H:3 * H]), _f32r(h0T[:, :RL]))
            rhn = cst.tile([H, RL], F32, name="rhn")
            nc.vector.scalar_tensor_tensor(
                out=rhn[:], in0=ps_hn[:], scalar=bhh_sb[:, 2:3], in1=rT[:],
                op0=ALU.add, op1=ALU.mult)
            tmp_n = cst.tile([H, RL], F32, name="tmp_n")
            nc.vector.tensor_add(tmp_n[:], ps_in[:], rhn[:])
            nT = cst.tile([H, RL], F32, name="nT")
            nc.scalar.activation(nT[:], tmp_n[:], ACT.Tanh, bias=bih_sb[:, 2:3])
            diff = cst.tile([H, RL], F32, name="diff")
            nc.vector.tensor_sub(diff[:], h0T[:, :RL], nT[:])
            zd = cst.tile([H, RL], F32, name="zd")
            nc.vector.tensor_mul(zd[:], zT[:], diff[:])
            h1T = cst.tile([H, RL], F32, name="h1T")
            nc.vector.tensor_add(h1T[:], nT[:], zd[:])

            # rich = hg[items] + h1; final = (rich + pos_emb[rev]) * colmask
            sgT = cst.tile([H, 512], BF16, name="sgT")
            gather_T(sgT, hg_full, items_sb, "gsg", dtype=BF16)
            poT = cst.tile([H, 512], F32, name="poT")
            gather_T(poT, posemb, rev_sb, "gpo")
            richT = cst.tile([H, RL], F32, name="richT")
            nc.vector.tensor_add(richT[:], h1T[:], sgT[:, :RL])
            finT = cst.tile([H, RL], F32, name="finT")
            nc.vector.tensor_add(finT[:], richT[:], poT[:, :RL])
            nc.vector.tensor_mul(finT[:], finT[:], cm_sb[:])

            # last[b] = final[b, len_b - 1]  (one-hot selection + reduce)
            lsel = cst.tile([H, RL], F32, name="lsel")
            nc.vector.tensor_mul(lsel[:], finT[:], ls_sb[:])
            lastT = cst.tile([H, NH], F32, name="lastT")
            nc.vector.reduce_sum(lastT[:], lsel[:].rearrange("p (b l) -> p b l", b=BLOC),
                                 axis=AX.X)

            # ---- multi-head attention (q = last, kv = final) ----
            qT = cst.tile([H, NH], F32, name="qT")
            ps_q = psum([H, NH])
            nc.tensor.matmul(ps_q[:], _f32r(prjT_sb[:, 0:H]), _f32r(lastT[:]))
            nc.scalar.activation(qT[:], ps_q[:], ACT.Identity, bias=prjb_sb[:, 0:1])
            kT = cst.tile([H, RL], F32, name="kT")
            ps_k = psum([H, RL])
            nc.tensor.matmul(ps_k[:], _f32r(prjT_sb[:, H:2 * H]), _f32r(finT[:]))
            nc.scalar.activation(kT[:], ps_k[:], ACT.Identity, bias=prjb_sb[:, 1:2])
            vT = cst.tile([H, RL], F32, name="vT")
            ps_v = psum([H, RL])
            nc.tensor.matmul(ps_v[:], _f32r(prjT_sb[:, 2 * H:3 * H]), _f32r(finT[:]))
            nc.scalar.activation(vT[:], ps_v[:], ACT.Identity, bias=prjb_sb[:, 2:3])

            ctxT = cst.tile([H, NH], F32, name="ctxT")
            for b in range(BLOC):
                qb = wk.tile([H, NH], F32, tag="qb")
                nc.vector.tensor_mul(qb[:], qT[:, b:b + 1].to_broadcast([H, NH]), bd_sb[:])
                ps_a = psum([NH, L], tag="ps")
                nc.tensor.matmul(ps_a[:], qb[:], kT[:, b * L:(b + 1) * L])
                attm = wk.tile([NH, L], F32, tag="attm")
                nc.vector.tensor_add(attm[:], ps_a[:], am_sb[:, b * L:(b + 1) * L])
                negmax = wk.tile([NH, 1], F32, tag="negmax")
                nc.vector.tensor_reduce(negmax[:], attm[:], axis=AX.X, op=ALU.max,
                                        negate=True)
                attE = wk.tile([NH, L], F32, tag="attE")
                den_a = wk.tile([NH, 1], F32, tag="den_a")
                nc.scalar.activation(attE[:], attm[:], ACT.Exp, bias=negmax[:, :1],
                                     accum_out=den_a[:, :1])
                rec_a = wk.tile([NH, 1], F32, tag="rec_a")
                nc.vector.reciprocal(rec_a[:], den_a[:])
                attw = wk.tile([NH, L], F32, tag="attw")
                nc.vector.tensor_scalar_mul(attw[:], attE[:], rec_a[:, :1])
                ps_wt = psum([L, NH])
                nc.tensor.transpose(ps_wt[:], attw[:], idn_sb[:NH, :NH])
                awT = wk.tile([L, NH], F32, tag="awT")
                nc.vector.tensor_copy(awT[:], ps_wt[:])
                ps_vt = psum([L, H])
                nc.tensor.transpose(ps_vt[:], vT[:, b * L:(b + 1) * L], idn_sb[:])
                vb = wk.tile([L, H], F32, tag="vb")
                nc.vector.tensor_copy(vb[:], ps_vt[:])
                ps_o = psum([H, NH], tag="ps")
                nc.tensor.matmul(ps_o[:], vb[:], awT[:])
                o2 = wk.tile([H, NH], F32, tag="o2")
                nc.vector.tensor_mul(o2[:], ps_o[:], bd_sb[:])
                nc.vector.reduce_sum(ctxT[:, b:b + 1], o2[:], axis=AX.X)

            sgloT = cst.tile([H, NH], F32, name="sgloT")
            ps_sg = psum([H, NH])
            nc.tensor.matmul(ps_sg[:], _f32r(oprjT_sb[:]), _f32r(ctxT[:]))
            nc.scalar.activation(sgloT[:], ps_sg[:], ACT.Identity, bias=oprjb_sb[:, :1])

            # ---- all-gather final / (last, s_global) ----
            nc.sync.dma_start(f_shard[:], finT[:])
            nc.gpsimd.collective_compute(
                "AllGather", ALU.bypass, replica_groups=[list(range(NC))],
                ins=[f_shard[:].opt()], outs=[f_full[:].opt()])
            lspair = cst.tile([H, 2 * NH], F32, name="lspair")
            nc.vector.tensor_copy(lspair[:, 0:NH], lastT[:])
            nc.vector.tensor_copy(lspair[:, NH:2 * NH], sgloT[:])
            nc.sync.dma_start(ls_shard[:], lspair[:])
            nc.gpsimd.collective_compute(
                "AllGather", ALU.bypass, replica_groups=[list(range(NC))],
                ins=[ls_shard[:].opt()], outs=[ls_full[:].opt()])

            fullT_f = cst.tile([H, B * L], F32, name="fullT_f")
            nc.sync.dma_start(fullT_f[:].rearrange("p (c r) -> p c r", c=NC),
                              f_full.rearrange("(c p) r -> p c r", p=H))
            fullT = cst.tile([H, B * L], F32R, name="fullT")
            nc.vector.tensor_copy(fullT[:], fullT_f[:])
            lastF_f = cst.tile([H, B], F32, name="lastF_f")
            sglF_f = cst.tile([H, B], F32, name="sglF_f")
            lsv = ls_full.rearrange("(c p) x -> p c x", p=H)
            nc.sync.dma_start(lastF_f[:].rearrange("p (c x) -> p c x", c=NC),
                              lsv[:, :, 0:NH])
            nc.sync.dma_start(sglF_f[:].rearrange("p (c x) -> p c x", c=NC),
                              lsv[:, :, NH:2 * NH])
            lastF = cst.tile([H, B], F32R, name="lastF")
            nc.vector.tensor_copy(lastF[:], lastF_f[:])
            sglF = cst.tile([H, B], F32R, name="sglF")
            nc.vector.tensor_copy(sglF[:], sglF_f[:])

            # =======================================================
            # Phase D: target attention over the candidate shard
            # =======================================================
            GW = RL  # 400 columns per b-group
            for ch in range(NCHUNK):
                num = wk.tile([H, B], F32, tag="num")
                den = wk.tile([H, B], F32, tag="den")
                eT = wk.tile([H, B * L], F32, tag="eT", bufs=2)
                pT = wk.tile([H, B * L], F32, tag="pT", bufs=2)
                for bg in range(BG):
                    rhs = fullT[:, bg * GW:(bg + 1) * GW]
                    ps_ts = psum([H, GW], tag="ts")
                    nc.tensor.matmul(ps_ts[:], _f32r(trT[:, ch * H:(ch + 1) * H]),
                                     _f32r(rhs))
                    ps_g = psum([H, GW], tag="gg")
                    nc.tensor.matmul(ps_g[:], _f32r(cT[0][:, ch * H:(ch + 1) * H]),
                                     _f32r(rhs))
                    nc.scalar.activation(eT[:, bg * GW:(bg + 1) * GW], ps_ts[:], ACT.Exp)
                    nc.vector.tensor_mul(pT[:, bg * GW:(bg + 1) * GW],
                                         eT[:, bg * GW:(bg + 1) * GW], ps_g[:])
                nc.vector.reduce_sum(den[:], eT[:].rearrange("p (b l) -> p b l", b=B),
                                     axis=AX.X)
                nc.vector.reduce_sum(num[:], pT[:].rearrange("p (b l) -> p b l", b=B),
                                     axis=AX.X)
                denf = wk.tile([H, B], F32, tag="denf")
                nc.vector.tensor_sub(denf[:], den[:], npad_sb[:])
                rec = wk.tile([H, B], F32, tag="rec")
                nc.vector.reciprocal(rec[:], denf[:])
                t1 = wk.tile([H, B], F32, tag="t1")
                nc.vector.tensor_mul(t1[:], num[:], rec[:])
                ps_23 = psum([H, B])
                nc.tensor.matmul(ps_23[:], _f32r(cT[1][:, ch * H:(ch + 1) * H]),
                                 _f32r(lastF[:]), start=True, stop=False)
                nc.tensor.matmul(ps_23[:], _f32r(cT[2][:, ch * H:(ch + 1) * H]),
                                 _f32r(sglF[:]), start=False, stop=True)
                outT = wk.tile([H, B], F32, tag="outT")
                nc.vector.tensor_add(outT[:], t1[:], ps_23[:])
                nc.sync.dma_start(scores_out[ch], outT[:])

    nc.compile()
    return nc


# ==============================================================
# Host side: shard inputs, run, gather output
# ==============================================================

def _prep(inputs):
    """Build per-core input maps (numpy only: layout/sharding/index prep)."""
    emb = np.asarray(inputs["emb"], np.float32)
    items = np.asarray(inputs["session_items"], np.int32)
    lens = np.asarray(inputs["session_len"], np.int32)
    adj = np.asarray(inputs["session_adj"], np.float32)
    erow = np.asarray(inputs["global_edge_row"], np.int32)
    ecol_g = np.asarray(inputs["global_edge_col"], np.int32)
    ew_g = np.asarray(inputs["global_edge_weight"], np.float32)

    rep = {}
    rep["embf"] = emb
    embb = emb.astype(ml_dtypes.bfloat16)
    rep["posemb"] = np.asarray(inputs["pos_emb"], np.float32)
    rep["idn"] = np.eye(H, dtype=np.float32)
    rep["iotab"] = np.broadcast_to(
        np.arange(WIN, dtype=np.float32), (H, WIN)).astype(ml_dtypes.bfloat16).copy()
    rep["blockdiag"] = np.kron(np.eye(NH, dtype=np.float32),
                               np.ones((H // NH, 1), np.float32))
    rep["w_lin_inT"] = np.ascontiguousarray(np.asarray(inputs["lin_in_W"], np.float32).T)
    rep["w_lin_outT"] = np.ascontiguousarray(np.asarray(inputs["lin_out_W"], np.float32).T)
    rep["b_lin_in"] = np.asarray(inputs["lin_in_b"], np.float32).reshape(H, 1)
    rep["b_lin_out"] = np.asarray(inputs["lin_out_b"], np.float32).reshape(H, 1)
    rep["w_ihT"] = np.ascontiguousarray(np.asarray(inputs["w_ih"], np.float32).T)
    rep["w_hhT"] = np.ascontiguousarray(np.asarray(inputs["w_hh"], np.float32).T)
    rep["b_ih"] = np.asarray(inputs["b_ih"], np.float32).reshape(3 * H, 1)
    rep["b_hh"] = np.asarray(inputs["b_hh"], np.float32).reshape(3 * H, 1)
    ipw = np.asarray(inputs["in_proj_w"], np.float32).copy()
    ipb = np.asarray(inputs["in_proj_b"], np.float32).copy()
    scale = 1.0 / math.sqrt(H // NH)
    ipw[:H] *= scale
    ipb[:H] *= scale
    rep["in_projT"] = np.ascontiguousarray(ipw.T)
    rep["in_projb"] = ipb.reshape(3 * H, 1)
    rep["out_projT"] = np.ascontiguousarray(np.asarray(inputs["out_proj_w"], np.float32).T)
    rep["out_projb"] = np.asarray(inputs["out_proj_b"], np.float32).reshape(H, 1)
    rep["gWT"] = np.ascontiguousarray(np.asarray(inputs["gW"], np.float32).T)
    rep["gb"] = np.asarray(inputs["gb"], np.float32).reshape(H, 1)
    rep["w3"] = np.asarray(inputs["w3_W"], np.float32)
    rep["wtT"] = np.ascontiguousarray(np.asarray(inputs["w_target_W"], np.float32).T)
    rep["npadr"] = np.broadcast_to((L - lens).astype(np.float32), (H, B)).copy()

    # --- global edges: sort by row, shard by vocab range, window-pack ---
    order = np.argsort(erow, kind="stable")
    erow_s, ecol_s, ew_s = erow[order], ecol_g[order], ew_g[order]
    # window id = row // WIN  (NC*NWIN = 40 windows over padded vocab)
    nwin_tot = NC * NWIN
    win_id = erow_s // WIN
    counts = np.bincount(win_id, minlength=nwin_tot)
    T = max(1, int(math.ceil(counts.max() / H)))
    starts = np.zeros(nwin_tot + 1, np.int64)
    np.cumsum(counts, out=starts[1:])

    cand_full = np.zeros((NPAD, H), np.float32)
    cand_full[:NIT - 1] = emb[1:]

    per_core = []
    for c in range(NC):
        ec = np.zeros((NWIN * T * H,), np.int32)
        er = np.full((NWIN * T * H,), 300.0, np.float32)
        evw = np.zeros((NWIN * T * H,), np.float32)
        for w in range(NWIN):
            gw = c * NWIN + w
            s, e = starts[gw], starts[gw + 1]
            n = e - s
            ec[w * T * H: w * T * H + n] = ecol_s[s:e]
            er[w * T * H: w * T * H + n] = (erow_s[s:e] - gw * WIN).astype(np.float32)
            evw[w * T * H: w * T * H + n] = ew_s[s:e]
        # [NWIN*T*H] -> [H, NWIN*T]: tile j, partition p <- j*H + p
        ec2 = ec.reshape(NWIN * T, H).T
        er2 = er.reshape(NWIN * T, H).T
        ev2 = evw.reshape(NWIN * T, H).T

        bsl = slice(c * BLOC, (c + 1) * BLOC)
        it_loc = items[bsl]                      # [8, 50]
        len_loc = lens[bsl]
        pos_idx = np.arange(L)[None, :]
        rev = len_loc[:, None] - 1 - pos_idx
        rev = np.where(it_loc == 0, 0, rev).astype(np.int32)
        pad = (it_loc == 0)

        itemsx = np.zeros((512, 1), np.int32)
        itemsx[:RL, 0] = it_loc.reshape(-1)
        revx = np.zeros((512, 1), np.int32)
        revx[:RL, 0] = rev.reshape(-1)
        attmask = np.where(pad, -1e9, 0.0).astype(np.float32).reshape(1, RL)
        colmask = (~pad).astype(np.float32).reshape(1, RL)
        lastsel = np.zeros((BLOC, L), np.float32)
        lastsel[np.arange(BLOC), len_loc - 1] = 1.0

        m = dict(rep)
        m["adjT"] = np.ascontiguousarray(adj[bsl].transpose(0, 2, 1))
        m["itemsx"] = itemsx
        m["revx"] = revx
        m["attmaskr"] = np.broadcast_to(attmask, (NH, RL)).copy()
        m["colmaskr"] = np.broadcast_to(colmask, (H, RL)).copy()
        m["lastselr"] = np.broadcast_to(lastsel.reshape(1, RL), (H, RL)).copy()
        m["candT"] = np.ascontiguousarray(cand_full[c * NS:(c + 1) * NS].T)
        m["eemb"] = np.ascontiguousarray(embb[ec2])
        m["erowrel"] = np.ascontiguousarray(er2)
        m["ew"] = np.ascontiguousarray(ev2)
        per_core.append(m)
    return per_core, T


def kernel(_trace=False, **inputs):
    in_maps, T = _prep(inputs)
    if T not in _NC_CACHE:
        _NC_CACHE[T] = build_nc(T)
    nc = _NC_CACHE[T]
    res = run_bass_kernel_spmd(nc, in_maps, core_ids=list(range(NC)),
                               trace=_trace)
    scores = np.concatenate(
        [res.results[c]["scoresT"].transpose(2, 0, 1).reshape(B, NS)
         for c in range(NC)], axis=1)[:, :NIT - 1]
    if _trace:
        return scores, res
    return scores


# revision 14
# speedup vs baseline: 3.2680x; 1.1078x over previous
"""Trainium2 Bass kernel for GCE-TAGNN session recommendation model.

Strategy:
  - Vocab axis (10000 items, padded to 10240 = 8*1280) sharded across 8 cores
    for the global sparse aggregation and the target-attention score/softmax.
  - Session path data-parallel: 8 sessions per core; final/last/s_global
    all-gathered so every core has the full batch for target attention.
  - Target attention reformulated: with d = cand @ w3_W  ([N,384]),
      scores[b,n] = (sum_l E[b,l,n]*g[b,l,n]) / (sum_l E[b,l,n])
                    + last[b]·d[n,128:256] + s_global[b]·d[n,256:384]
    where ts[b,l,n] = final[b,l]·(w_target_W @ cand[n]), E = exp(ts) (no max
    subtraction needed: |ts| is small), g[b,l,n] = final[b,l]·d[n,:128].
    Padded (b,l) columns of final are zeroed, so E=1 and g=0 there; the
    softmax denominator is corrected by subtracting (L - len[b]).
"""

import sys

sys.path.insert(0, "/opt/trn_rl_repo")

import math

import ml_dtypes
import numpy as np

import concourse.bass as bass
import concourse.mybir as mybir
import concourse.tile as tile
from concourse import bacc
from concourse.bass import IndirectOffsetOnAxis
from concourse.bass_utils import run_bass_kernel_spmd

F32 = mybir.dt.float32
F32R = mybir.dt.float32r
BF16 = mybir.dt.bfloat16
I32 = mybir.dt.int32
AX = mybir.AxisListType
ALU = mybir.AluOpType
ACT = mybir.ActivationFunctionType

NC = 8          # cores
B = 64          # batch
L = 50          # session length
H = 128         # hidden
NH = 8          # heads
NIT = 10000     # item vocab
NPAD = NC * 1280  # padded vocab for candidate sharding
NS = 1280       # candidate shard per core
NCHUNK = NS // 128  # 10 n-chunks of 128 per core
BLOC = B // NC  # sessions per core
RL = BLOC * L   # 400 rows per core
WIN = 128       # agg row window
NWIN = 1280 // WIN  # 5 windows per core
BG = 8          # b-groups in phase D (each BLOC sessions = 400 cols)

USE_F32R = True


def _f32r(ap):
    return ap


_NC_CACHE = {}


def build_nc(T):
    """Build the single-NEFF SPMD program. T = edge tiles per window."""
    nc = bacc.Bacc(None, target_bir_lowering=False)

    def inp(name, shape, dtype=F32):
        return nc.dram_tensor(name, shape, dtype, kind="ExternalInput")

    # ---- replicated weights/constants ----
    embf = inp("embf", [NIT, H])
    posemb = inp("posemb", [200, H])
    idn = inp("idn", [H, H])
    iotab = inp("iotab", [H, WIN], BF16)
    blockdiag = inp("blockdiag", [H, NH])
    w_lin_inT = inp("w_lin_inT", [H, H])
    w_lin_outT = inp("w_lin_outT", [H, H])
    b_lin_in = inp("b_lin_in", [H, 1])
    b_lin_out = inp("b_lin_out", [H, 1])
    w_ihT = inp("w_ihT", [2 * H, 3 * H])
    w_hhT = inp("w_hhT", [H, 3 * H])
    b_ih = inp("b_ih", [3 * H, 1])
    b_hh = inp("b_hh", [3 * H, 1])
    in_projT = inp("in_projT", [H, 3 * H])
    in_projb = inp("in_projb", [3 * H, 1])
    out_projT = inp("out_projT", [H, H])
    out_projb = inp("out_projb", [H, 1])
    gWT = inp("gWT", [H, H])
    gb = inp("gb", [H, 1])
    w3 = inp("w3", [H, 3 * H])
    wtT = inp("wtT", [H, H])
    npadr = inp("npadr", [H, B])
    # ---- per-core ----
    adjT = inp("adjT", [BLOC, L, L])
    itemsx = inp("itemsx", [512, 1], I32)
    revx = inp("revx", [512, 1], I32)
    attmaskr = inp("attmaskr", [NH, RL])
    colmaskr = inp("colmaskr", [H, RL])
    lastselr = inp("lastselr", [H, RL])
    candT = inp("candT", [H, NS])
    eemb = inp("eemb", [H, NWIN * T, H], BF16)
    erowrel = inp("erowrel", [H, NWIN * T])
    ew = inp("ew", [H, NWIN * T])

    scores_out = nc.dram_tensor("scoresT", [NCHUNK, H, B], F32, kind="ExternalOutput")

    with tile.TileContext(nc) as tc:
        with (
            tc.tile_pool(name="cst", bufs=1) as cst,
            tc.tile_pool(name="wk", bufs=3) as wk,
            tc.tile_pool(name="pp", bufs=8, space="PSUM") as pp,
            tc.tile_pool(name="dr", bufs=1, space="DRAM") as dr,
        ):
            def psum(shape, tag="ps"):
                nbuf = {"ps": 3, "ts": 2, "gg": 2}[tag]
                return pp.tile(shape, F32, tag=tag, name=tag, bufs=nbuf)

            # ---------- load constants into SBUF ----------
            def load(name, src, shape=None, dtype=F32):
                t = cst.tile(shape if shape is not None else src.shape, dtype, name=name)
                nc.sync.dma_start(t[:], src[:])
                return t

            idn_sb = load("idn_sb", idn)
            idnb_sb = cst.tile([H, H], BF16, name="idnb_sb")
            nc.vector.tensor_copy(idnb_sb[:], idn_sb[:])
            iota_sb = load("iota_sb", iotab, dtype=BF16)
            bd_sb = load("bd_sb", blockdiag)
            linT_sb = load("linT_sb", w_lin_inT)
            loutT_sb = load("loutT_sb", w_lin_outT)
            blin_sb = load("blin_sb", b_lin_in)
            blout_sb = load("blout_sb", b_lin_out)
            wih_sb = cst.tile([H, 2, 3 * H], F32, name="wih_sb")
            nc.sync.dma_start(wih_sb[:], w_ihT.rearrange("(a p) j -> p a j", p=H))
            whh_sb = load("whh_sb", w_hhT)
            bih_sb = load("bih_sb", b_ih, shape=[H, 3])   # [384,1] -> [128,3]
            bhh_sb = load("bhh_sb", b_hh, shape=[H, 3])
            # reinterpret [384,1] dram as [128,3]: partition p, col g -> b[g*128+p]
            nc.sync.dma_start(bih_sb[:], b_ih.rearrange("(g p) o -> p (g o)", p=H))
            nc.sync.dma_start(bhh_sb[:], b_hh.rearrange("(g p) o -> p (g o)", p=H))
            prjT_sb = load("prjT_sb", in_projT)
            prjb_sb = cst.tile([H, 3], F32, name="prjb_sb")
            nc.sync.dma_start(prjb_sb[:], in_projb.rearrange("(g p) o -> p (g o)", p=H))
            oprjT_sb = load("oprjT_sb", out_projT)
            oprjb_sb = load("oprjb_sb", out_projb)
            gWT_f = load("gWT_f", gWT)
            gWT_sb = cst.tile([H, H], F32R, name="gWT_sb")
            nc.vector.tensor_copy(gWT_sb[:], gWT_f[:])
            gb_sb = load("gb_sb", gb)
            w3_f = load("w3_f", w3)
            w3_sb = cst.tile([H, 3 * H], F32R, name="w3_sb")
            nc.vector.tensor_copy(w3_sb[:], w3_f[:])
            wtT_f = load("wtT_f", wtT)
            wtT_sb = cst.tile([H, H], F32R, name="wtT_sb")
            nc.vector.tensor_copy(wtT_sb[:], wtT_f[:])
            npad_sb = load("npad_sb", npadr)
            am_sb = load("am_sb", attmaskr)
            cm_sb = load("cm_sb", colmaskr)
            ls_sb = load("ls_sb", lastselr)
            candT_f = load("candT_f", candT)
            candT_sb = cst.tile([H, NS], F32R, name="candT_sb")
            nc.vector.tensor_copy(candT_sb[:], candT_f[:])
            erow_sb = load("erow_sb", erowrel)
            ew_sb = load("ew_sb", ew)
            items_sb = cst.tile([H, 4], I32, name="items_sb")
            nc.sync.dma_start(items_sb[:], itemsx.rearrange("(t p) o -> p (t o)", p=H))
            rev_sb = cst.tile([H, 4], I32, name="rev_sb")
            nc.sync.dma_start(rev_sb[:], revx.rearrange("(t p) o -> p (t o)", p=H))

            # DRAM bounce buffers for collectives
            hg_shard = dr.tile([NS, H], BF16, name="hg_shard")
            hg_full = dr.tile([NC * NS, H], BF16, addr_space="Shared", name="hg_full")
            f_shard = dr.tile([H, RL], F32, name="f_shard")
            f_full = dr.tile([NC * H, RL], F32, addr_space="Shared", name="f_full")
            ls_shard = dr.tile([H, 2 * NH], F32, name="ls_shard")
            ls_full = dr.tile([NC * H, 2 * NH], F32, addr_space="Shared", name="ls_full")

            # =======================================================
            # Phase C: candidate transforms (independent of all else)
            # =======================================================
            cT = [cst.tile([H, NS], F32R, name=f"c{j}T") for j in range(3)]
            trT = cst.tile([H, NS], F32R, name="trT")
            nblk = [(0, 512), (512, 512), (1024, 256)]
            for j in range(3):
                for off, w in nblk:
                    ps = psum([H, w])
                    nc.tensor.matmul(
                        ps[:], _f32r(w3_sb[:, j * H:(j + 1) * H]),
                        _f32r(candT_sb[:, off:off + w]))
                    nc.vector.tensor_copy(cT[j][:, off:off + w], ps[:])
            for off, w in nblk:
                ps = psum([H, w])
                nc.tensor.matmul(ps[:], _f32r(wtT_sb[:]), _f32r(candT_sb[:, off:off + w]))
                nc.vector.tensor_copy(trT[:, off:off + w], ps[:])

            # =======================================================
            # Phase A: global GNN aggregation (vocab shard, 5 windows)
            # =======================================================
            aggT = cst.tile([H, NS], F32R, name="aggT")
            for w in range(NWIN):
                mt = wk.tile([H, T, H], BF16, tag="mt", bufs=2)
                nc.sync.dma_start(mt[:], eemb[:, w * T:(w + 1) * T, :])
                agg_ps = psum([H, WIN])
                for t in range(T):
                    j = w * T + t
                    sw = wk.tile([H, WIN], BF16, tag="sw")
                    nc.vector.tensor_scalar(
                        out=sw[:], in0=iota_sb[:], scalar1=erow_sb[:, j:j + 1],
                        scalar2=ew_sb[:, j:j + 1], op0=ALU.is_equal, op1=ALU.mult)
                    nc.tensor.matmul(agg_ps[:], mt[:, t, :], sw[:],
                                     start=(t == 0), stop=(t == T - 1))
                nc.vector.tensor_copy(aggT[:, w * WIN:(w + 1) * WIN], agg_ps[:])
            # hgT = relu(gW @ agg + gb), stored bf16 for a cheaper all-gather
            hgT = cst.tile([H, NS], BF16, name="hgT")
            for off, w in nblk:
                ps = psum([H, w])
                nc.tensor.matmul(ps[:], _f32r(gWT_sb[:]), _f32r(aggT[:, off:off + w]))
                nc.scalar.activation(hgT[:, off:off + w], ps[:], ACT.Relu, bias=gb_sb[:, :1])
            # transpose to row-major [1280, 128] and store for all-gather
            hg_rm = cst.tile([H, NCHUNK, H], BF16, name="hg_rm")
            for k in range(NCHUNK):
                ps_b = pp.tile([H, H], BF16, tag="ps", name="ps_b", bufs=3)
                nc.tensor.transpose(ps_b[:], hgT[:, k * H:(k + 1) * H], idnb_sb[:])
                nc.vector.tensor_copy(hg_rm[:, k, :], ps_b[:])
            nc.sync.dma_start(hg_shard.rearrange("(k p) h -> p k h", p=H), hg_rm[:])
            nc.gpsimd.collective_compute(
                "AllGather", ALU.bypass, replica_groups=[list(range(NC))],
                ins=[hg_shard[:].opt()], outs=[hg_full[:].opt()])

            # =======================================================
            # Phase B: session path (8 local sessions)
            # =======================================================
            def gather_T(dst, table, idx_sb, tag, dtype=F32):
                """gather rows table[idx] -> transpose -> dst [128, 512]."""
                for t in range(4):
                    g = wk.tile([H, H], dtype, tag=tag)
                    nc.gpsimd.indirect_dma_start(
                        out=g[:], out_offset=None, in_=table[:, :],
                        in_offset=IndirectOffsetOnAxis(ap=idx_sb[:, t:t + 1], axis=0))
                    if dtype == BF16:
                        ps_g2 = pp.tile([H, H], BF16, tag="ps", name="ps_g2", bufs=3)
                        nc.tensor.transpose(ps_g2[:], g[:], idnb_sb[:])
                        nc.vector.tensor_copy(dst[:, t * H:(t + 1) * H], ps_g2[:])
                    else:
                        ps = psum([H, H])
                        nc.tensor.transpose(ps[:], g[:], idn_sb[:])
                        nc.vector.tensor_copy(dst[:, t * H:(t + 1) * H], ps[:])

            h0T = cst.tile([H, 512], F32, name="h0T")
            gather_T(h0T, embf, items_sb, "gh0")

            # Y = lin(h);  inp = adj @ Y   (per session)
            yinT = cst.tile([H, RL], F32, name="yinT")
            youtT = cst.tile([H, RL], F32, name="youtT")
            ps = psum([H, RL])
            nc.tensor.matmul(ps[:], _f32r(linT_sb[:]), _f32r(h0T[:, :RL]))
            nc.scalar.activation(yinT[:], ps[:], ACT.Identity, bias=blin_sb[:, :1])
            ps = psum([H, RL])
            nc.tensor.matmul(ps[:], _f32r(loutT_sb[:]), _f32r(h0T[:, :RL]))
            nc.scalar.activation(youtT[:], ps[:], ACT.Identity, bias=blout_sb[:, :1])

            iinT = cst.tile([H, RL], F32, name="iinT")
            ioutT = cst.tile([H, RL], F32, name="ioutT")
            for b in range(BLOC):
                at = wk.tile([L, L], F32, tag="at")
                nc.sync.dma_start(at[:], adjT[b])
                for yT, dst in ((yinT, iinT), (youtT, ioutT)):
                    ps_t = psum([L, H])
                    nc.tensor.transpose(ps_t[:], yT[:, b * L:(b + 1) * L], idn_sb[:])
                    yb = wk.tile([L, H], F32, tag="yb")
                    nc.vector.tensor_copy(yb[:], ps_t[:])
                    ps_i = psum([H, L], tag="ps")
                    nc.tensor.matmul(ps_i[:], yb[:], at[:])
                    nc.vector.tensor_copy(dst[:, b * L:(b + 1) * L], ps_i[:])

            # GRU cell (feature-major)
            combR = cst.tile([H, 2], F32, name="combR")
            nc.vector.tensor_add(combR[:, 0:1], bih_sb[:, 0:1], bhh_sb[:, 0:1])
            nc.vector.tensor_add(combR[:, 1:2], bih_sb[:, 1:2], bhh_sb[:, 1:2])
            gates = []
            for g in range(2):  # r, z
                ps_g = psum([H, RL])
                nc.tensor.matmul(ps_g[:], _f32r(wih_sb[:, 0, g * H:(g + 1) * H]),
                                 _f32r(iinT[:]), start=True, stop=False)
                nc.tensor.matmul(ps_g[:], _f32r(wih_sb[:, 1, g * H:(g + 1) * H]),
                                 _f32r(ioutT[:]), start=False, stop=False)
                nc.tensor.matmul(ps_g[:], _f32r(whh_sb[:, g * H:(g + 1) * H]),
                                 _f32r(h0T[:, :RL]), start=False, stop=True)
                gt = cst.tile([H, RL], F32, name=f"gate{g}")
                nc.scalar.activation(gt[:], ps_g[:], ACT.Sigmoid, bias=combR[:, g:g + 1])
                gates.append(gt)
            rT, zT = gates
            ps_in = psum([H, RL])
            nc.tensor.matmul(ps_in[:], _f32r(wih_sb[:, 0, 2 * H:3 * H]), _f32r(iinT[:]),
                             start=True, stop=False)
            nc.tensor.matmul(ps_in[:], _f32r(wih_sb[:, 1, 2 * H:3 * H]), _f32r(ioutT[:]),
                             start=False, stop=True)
            ps_hn = psum([H, RL])
            nc.tensor.matmul(ps_hn[:], _f32r(whh_sb[:, 2 * H:3 * H]), _f32r(h0T[:, :RL]))
            rhn = cst.tile([H, RL], F32, name="rhn")
            nc.vector.scalar_tensor_tensor(
                out=rhn[:], in0=ps_hn[:], scalar=bhh_sb[:, 2:3], in1=rT[:],
                op0=ALU.add, op1=ALU.mult)
            tmp_n = cst.tile([H, RL], F32, name="tmp_n")
            nc.vector.tensor_add(tmp_n[:], ps_in[:], rhn[:])
            nT = cst.tile([H, RL], F32, name="nT")
            nc.scalar.activation(nT[:], tmp_n[:], ACT.Tanh, bias=bih_sb[:, 2:3])
            diff = cst.tile([H, RL], F32, name="diff")
            nc.vector.tensor_sub(diff[:], h0T[:, :RL], nT[:])
            zd = cst.tile([H, RL], F32, name="zd")
            nc.vector.tensor_mul(zd[:], zT[:], diff[:])
            h1T = cst.tile([H, RL], F32, name="h1T")
            nc.vector.tensor_add(h1T[:], nT[:], zd[:])

            # rich = hg[items] + h1; final = (rich + pos_emb[rev]) * colmask
            sgT = cst.tile([H, 512], BF16, name="sgT")
            gather_T(sgT, hg_full, items_sb, "gsg", dtype=BF16)
            poT = cst.tile([H, 512], F32, name="poT")
            gather_T(poT, posemb, rev_sb, "gpo")
            richT = cst.tile([H, RL], F32, name="richT")
            nc.vector.tensor_add(richT[:], h1T[:], sgT[:, :RL])
            finT = cst.tile([H, RL], F32, name="finT")
            nc.vector.tensor_add(finT[:], richT[:], poT[:, :RL])
            nc.vector.tensor_mul(finT[:], finT[:], cm_sb[:])

            # last[b] = final[b, len_b - 1]  (one-hot selection + reduce)
            lsel = cst.tile([H, RL], F32, name="lsel")
            nc.vector.tensor_mul(lsel[:], finT[:], ls_sb[:])
            lastT = cst.tile([H, NH], F32, name="lastT")
            nc.vector.reduce_sum(lastT[:], lsel[:].rearrange("p (b l) -> p b l", b=BLOC),
                                 axis=AX.X)

            # ---- multi-head attention (q = last, kv = final) ----
            qT = cst.tile([H, NH], F32, name="qT")
            ps_q = psum([H, NH])
            nc.tensor.matmul(ps_q[:], _f32r(prjT_sb[:, 0:H]), _f32r(lastT[:]))
            nc.scalar.activation(qT[:], ps_q[:], ACT.Identity, bias=prjb_sb[:, 0:1])
            kT = cst.tile([H, RL], F32, name="kT")
            ps_k = psum([H, RL])
            nc.tensor.matmul(ps_k[:], _f32r(prjT_sb[:, H:2 * H]), _f32r(finT[:]))
            nc.scalar.activation(kT[:], ps_k[:], ACT.Identity, bias=prjb_sb[:, 1:2])
            vT = cst.tile([H, RL], F32, name="vT")
            ps_v = psum([H, RL])
            nc.tensor.matmul(ps_v[:], _f32r(prjT_sb[:, 2 * H:3 * H]), _f32r(finT[:]))
            nc.scalar.activation(vT[:], ps_v[:], ACT.Identity, bias=prjb_sb[:, 2:3])

            ctxT = cst.tile([H, NH], F32, name="ctxT")
            for b in range(BLOC):
                qb = wk.tile([H, NH], F32, tag="qb")
                nc.vector.tensor_mul(qb[:], qT[:, b:b + 1].to_broadcast([H, NH]), bd_sb[:])
                ps_a = psum([NH, L], tag="ps")
                nc.tensor.matmul(ps_a[:], qb[:], kT[:, b * L:(b + 1) * L])
                attm = wk.tile([NH, L], F32, tag="attm")
                nc.vector.tensor_add(attm[:], ps_a[:], am_sb[:, b * L:(b + 1) * L])
                negmax = wk.tile([NH, 1], F32, tag="negmax")
                nc.vector.tensor_reduce(negmax[:], attm[:], axis=AX.X, op=ALU.max,
                                        negate=True)
                attE = wk.tile([NH, L], F32, tag="attE")
                den_a = wk.tile([NH, 1], F32, tag="den_a")
                nc.scalar.activation(attE[:], attm[:], ACT.Exp, bias=negmax[:, :1],
                                     accum_out=den_a[:, :1])
                rec_a = wk.tile([NH, 1], F32, tag="rec_a")
                nc.vector.reciprocal(rec_a[:], den_a[:])
                attw = wk.tile([NH, L], F32, tag="attw")
                nc.vector.tensor_scalar_mul(attw[:], attE[:], rec_a[:, :1])
                ps_wt = psum([L, NH])
                nc.tensor.transpose(ps_wt[:], attw[:], idn_sb[:NH, :NH])
                awT = wk.tile([L, NH], F32, tag="awT")
                nc.vector.tensor_copy(awT[:], ps_wt[:])
                ps_vt = psum([L, H])
                nc.tensor.transpose(ps_vt[:], vT[:, b * L:(b + 1) * L], idn_sb[:])
                vb = wk.tile([L, H], F32, tag="vb")
                nc.vector.tensor_copy(vb[:], ps_vt[:])
                ps_o = psum([H, NH], tag="ps")
                nc.tensor.matmul(ps_o[:], vb[:], awT[:])
                o2 = wk.tile([H, NH], F32, tag="o2")
                nc.vector.tensor_mul(o2[:], ps_o[:], bd_sb[:])
                nc.vector.reduce_sum(ctxT[:, b:b + 1], o2[:], axis=AX.X)

            sgloT = cst.tile([H, NH], F32, name="sgloT")
            ps_sg = psum([H, NH])
            nc.tensor.matmul(ps_sg[:], _f32r(oprjT_sb[:]), _f32r(ctxT[:]))
            nc.scalar.activation(sgloT[:], ps_sg[:], ACT.Identity, bias=oprjb_sb[:, :1])

            # ---- all-gather final / (last, s_global) ----
            nc.sync.dma_start(f_shard[:], finT[:])
            nc.gpsimd.collective_compute(
                "AllGather", ALU.bypass, replica_groups=[list(range(NC))],
                ins=[f_shard[:].opt()], outs=[f_full[:].opt()])
            lspair = cst.tile([H, 2 * NH], F32, name="lspair")
            nc.vector.tensor_copy(lspair[:, 0:NH], lastT[:])
            nc.vector.tensor_copy(lspair[:, NH:2 * NH], sgloT[:])
            nc.sync.dma_start(ls_shard[:], lspair[:])
            nc.gpsimd.collective_compute(
                "AllGather", ALU.bypass, replica_groups=[list(range(NC))],
                ins=[ls_shard[:].opt()], outs=[ls_full[:].opt()])

            fullT_f = cst.tile([H, B * L], F32, name="fullT_f")
            nc.sync.dma_start(fullT_f[:].rearrange("p (c r) -> p c r", c=NC),
                              f_full.rearrange("(c p) r -> p c r", p=H))
            fullT = cst.tile([H, B * L], F32R, name="fullT")
            nc.vector.tensor_copy(fullT[:], fullT_f[:])
            lastF_f = cst.tile([H, B], F32, name="lastF_f")
            sglF_f = cst.tile([H, B], F32, name="sglF_f")
            lsv = ls_full.rearrange("(c p) x -> p c x", p=H)
            nc.sync.dma_start(lastF_f[:].rearrange("p (c x) -> p c x", c=NC),
                              lsv[:, :, 0:NH])
            nc.sync.dma_start(sglF_f[:].rearrange("p (c x) -> p c x", c=NC),
                              lsv[:, :, NH:2 * NH])
            lastF = cst.tile([H, B], F32R, name="lastF")
            nc.vector.tensor_copy(lastF[:], lastF_f[:])
            sglF = cst.tile([H, B], F32R, name="sglF")
            nc.vector.tensor_copy(sglF[:], sglF_f[:])

            # =======================================================
            # Phase D: target attention over the candidate shard
            # =======================================================
            GW = RL  # 400 columns per b-group
            for ch in range(NCHUNK):
                num = wk.tile([H, B], F32, tag="num")
                den = wk.tile([H, B], F32, tag="den")
                eT = wk.tile([H, B * L], F32, tag="eT", bufs=2)
                pT = wk.tile([H, B * L], F32, tag="pT", bufs=2)
                for bg in range(BG):
                    rhs = fullT[:, bg * GW:(bg + 1) * GW]
                    ps_ts = psum([H, GW], tag="ts")
                    nc.tensor.matmul(ps_ts[:], _f32r(trT[:, ch * H:(ch + 1) * H]),
                                     _f32r(rhs))
                    ps_g = psum([H, GW], tag="gg")
                    nc.tensor.matmul(ps_g[:], _f32r(cT[0][:, ch * H:(ch + 1) * H]),
                                     _f32r(rhs))
                    nc.scalar.activation(eT[:, bg * GW:(bg + 1) * GW], ps_ts[:], ACT.Exp)
                    nc.vector.tensor_mul(pT[:, bg * GW:(bg + 1) * GW],
                                         eT[:, bg * GW:(bg + 1) * GW], ps_g[:])
                    if bg % 2 == 1:
                        o, w2 = (bg - 1) * GW, 2 * GW
                        ob, wb = (bg - 1) * BLOC, 2 * BLOC
                        nc.vector.reduce_sum(
                            den[:, ob:ob + wb],
                            eT[:, o:o + w2].rearrange("p (b l) -> p b l", b=wb),
                            axis=AX.X)
                        nc.vector.reduce_sum(
                            num[:, ob:ob + wb],
                            pT[:, o:o + w2].rearrange("p (b l) -> p b l", b=wb),
                            axis=AX.X)
                denf = wk.tile([H, B], F32, tag="denf")
                nc.vector.tensor_sub(denf[:], den[:], npad_sb[:])
                rec = wk.tile([H, B], F32, tag="rec")
                nc.vector.reciprocal(rec[:], denf[:])
                t1 = wk.tile([H, B], F32, tag="t1")
                nc.vector.tensor_mul(t1[:], num[:], rec[:])
                ps_23 = psum([H, B])
                nc.tensor.matmul(ps_23[:], _f32r(cT[1][:, ch * H:(ch + 1) * H]),
                                 _f32r(lastF[:]), start=True, stop=False)
                nc.tensor.matmul(ps_23[:], _f32r(cT[2][:, ch * H:(ch + 1) * H]),
                                 _f32r(sglF[:]), start=False, stop=True)
                outT = wk.tile([H, B], F32, tag="outT")
                nc.vector.tensor_add(outT[:], t1[:], ps_23[:])
                nc.sync.dma_start(scores_out[ch], outT[:])

    nc.compile()
    return nc


# ==============================================================
# Host side: shard inputs, run, gather output
# ==============================================================

def _prep(inputs):
    """Build per-core input maps (numpy only: layout/sharding/index prep)."""
    emb = np.asarray(inputs["emb"], np.float32)
    items = np.asarray(inputs["session_items"], np.int32)
    lens = np.asarray(inputs["session_len"], np.int32)
    adj = np.asarray(inputs["session_adj"], np.float32)
    erow = np.asarray(inputs["global_edge_row"], np.int32)
    ecol_g = np.asarray(inputs["global_edge_col"], np.int32)
    ew_g = np.asarray(inputs["global_edge_weight"], np.float32)

    rep = {}
    rep["embf"] = emb
    embb = emb.astype(ml_dtypes.bfloat16)
    rep["posemb"] = np.asarray(inputs["pos_emb"], np.float32)
    rep["idn"] = np.eye(H, dtype=np.float32)
    rep["iotab"] = np.broadcast_to(
        np.arange(WIN, dtype=np.float32), (H, WIN)).astype(ml_dtypes.bfloat16).copy()
    rep["blockdiag"] = np.kron(np.eye(NH, dtype=np.float32),
                               np.ones((H // NH, 1), np.float32))
    rep["w_lin_inT"] = np.ascontiguousarray(np.asarray(inputs["lin_in_W"], np.float32).T)
    rep["w_lin_outT"] = np.ascontiguousarray(np.asarray(inputs["lin_out_W"], np.float32).T)
    rep["b_lin_in"] = np.asarray(inputs["lin_in_b"], np.float32).reshape(H, 1)
    rep["b_lin_out"] = np.asarray(inputs["lin_out_b"], np.float32).reshape(H, 1)
    rep["w_ihT"] = np.ascontiguousarray(np.asarray(inputs["w_ih"], np.float32).T)
    rep["w_hhT"] = np.ascontiguousarray(np.asarray(inputs["w_hh"], np.float32).T)
    rep["b_ih"] = np.asarray(inputs["b_ih"], np.float32).reshape(3 * H, 1)
    rep["b_hh"] = np.asarray(inputs["b_hh"], np.float32).reshape(3 * H, 1)
    ipw = np.asarray(inputs["in_proj_w"], np.float32).copy()
    ipb = np.asarray(inputs["in_proj_b"], np.float32).copy()
    scale = 1.0 / math.sqrt(H // NH)
    ipw[:H] *= scale
    ipb[:H] *= scale
    rep["in_projT"] = np.ascontiguousarray(ipw.T)
    rep["in_projb"] = ipb.reshape(3 * H, 1)
    rep["out_projT"] = np.ascontiguousarray(np.asarray(inputs["out_proj_w"], np.float32).T)
    rep["out_projb"] = np.asarray(inputs["out_proj_b"], np.float32).reshape(H, 1)
    rep["gWT"] = np.ascontiguousarray(np.asarray(inputs["gW"], np.float32).T)
    rep["gb"] = np.asarray(inputs["gb"], np.float32).reshape(H, 1)
    rep["w3"] = np.asarray(inputs["w3_W"], np.float32)
    rep["wtT"] = np.ascontiguousarray(np.asarray(inputs["w_target_W"], np.float32).T)
    rep["npadr"] = np.broadcast_to((L - lens).astype(np.float32), (H, B)).copy()

    # --- global edges: sort by row, shard by vocab range, window-pack ---
    order = np.argsort(erow, kind="stable")
    erow_s, ecol_s, ew_s = erow[order], ecol_g[order], ew_g[order]
    # window id = row // WIN  (NC*NWIN = 40 windows over padded vocab)
    nwin_tot = NC * NWIN
    win_id = erow_s // WIN
    counts = np.bincount(win_id, minlength=nwin_tot)
    T = max(1, int(math.ceil(counts.max() / H)))
    starts = np.zeros(nwin_tot + 1, np.int64)
    np.cumsum(counts, out=starts[1:])

    cand_full = np.zeros((NPAD, H), np.float32)
    cand_full[:NIT - 1] = emb[1:]

    per_core = []
    for c in range(NC):
        ec = np.zeros((NWIN * T * H,), np.int32)
        er = np.full((NWIN * T * H,), 300.0, np.float32)
        evw = np.zeros((NWIN * T * H,), np.float32)
        for w in range(NWIN):
            gw = c * NWIN + w
            s, e = starts[gw], starts[gw + 1]
            n = e - s
            ec[w * T * H: w * T * H + n] = ecol_s[s:e]
            er[w * T * H: w * T * H + n] = (erow_s[s:e] - gw * WIN).astype(np.float32)
            evw[w * T * H: w * T * H + n] = ew_s[s:e]
        # [NWIN*T*H] -> [H, NWIN*T]: tile j, partition p <- j*H + p
        ec2 = ec.reshape(NWIN * T, H).T
        er2 = er.reshape(NWIN * T, H).T
        ev2 = evw.reshape(NWIN * T, H).T

        bsl = slice(c * BLOC, (c + 1) * BLOC)
        it_loc = items[bsl]                      # [8, 50]
        len_loc = lens[bsl]
        pos_idx = np.arange(L)[None, :]
        rev = len_loc[:, None] - 1 - pos_idx
        rev = np.where(it_loc == 0, 0, rev).astype(np.int32)
        pad = (it_loc == 0)

        itemsx = np.zeros((512, 1), np.int32)
        itemsx[:RL, 0] = it_loc.reshape(-1)
        revx = np.zeros((512, 1), np.int32)
        revx[:RL, 0] = rev.reshape(-1)
        attmask = np.where(pad, -1e9, 0.0).astype(np.float32).reshape(1, RL)
        colmask = (~pad).astype(np.float32).reshape(1, RL)
        lastsel = np.zeros((BLOC, L), np.float32)
        lastsel[np.arange(BLOC), len_loc - 1] = 1.0

        m = dict(rep)
        m["adjT"] = np.ascontiguousarray(adj[bsl].transpose(0, 2, 1))
        m["itemsx"] = itemsx
        m["revx"] = revx
        m["attmaskr"] = np.broadcast_to(attmask, (NH, RL)).copy()
        m["colmaskr"] = np.broadcast_to(colmask, (H, RL)).copy()
        m["lastselr"] = np.broadcast_to(lastsel.reshape(1, RL), (H, RL)).copy()
        m["candT"] = np.ascontiguousarray(cand_full[c * NS:(c + 1) * NS].T)
        m["eemb"] = np.ascontiguousarray(embb[ec2])
        m["erowrel"] = np.ascontiguousarray(er2)
        m["ew"] = np.ascontiguousarray(ev2)
        per_core.append(m)
    return per_core, T


def kernel(_trace=False, **inputs):
    in_maps, T = _prep(inputs)
    if T not in _NC_CACHE:
        _NC_CACHE[T] = build_nc(T)
    nc = _NC_CACHE[T]
    res = run_bass_kernel_spmd(nc, in_maps, core_ids=list(range(NC)),
                               trace=_trace)
    scores = np.concatenate(
        [res.results[c]["scoresT"].transpose(2, 0, 1).reshape(B, NS)
         for c in range(NC)], axis=1)[:, :NIT - 1]
    if _trace:
        return scores, res
    return scores
